# revision 9
# baseline (speedup 1.0000x reference)
"""DriftAwareMultiHeadAttention on 8 Trainium2 NeuronCores.

Sharding (per spec hint): core c -> (batch b = c//2, head-group hg = c%2).
Each core runs the QKV projection column-parallel over its 8 heads, full
attention for those heads, and a row-parallel partial output projection.
Host gather: y[b] = (yT[2b] + yT[2b+1]).T + b_out.

v2: paired-head phase 2.  The per-core 8 heads are processed as 4 pairs
(2j, 2j+1); the pair's K^T/Q^T live in the same e-tile at partition
offsets 0/64, so their K=64 score matmuls carry tile_position (0,0) and
(64,0) and run CONCURRENTLY on disjoint row-groups of the PE array
(2 matmuls per ~N cycles instead of 1).  This removes the half-array
waste of the hd=64 contraction: scores cost ~56us instead of ~110us.

Device layout is feature-on-partition / token-on-free throughout:
  - Q^T, K^T: [512, 2048] fp16 (pair j -> e-tile j, head parity ->
    partition offset 0/64).  V: [tokens, 8 heads x (64+1)] fp16 -- the
    extra "ones" column makes the AV matmul emit the softmax denominator
    in psum row 64 for free.
  - Unit (qc, j): 32 cells (cell = kt*2 + parity) of S^T score matmuls,
    emitted in cell order so adjacent matmuls alternate row-groups.
    Segments of 3 cells -> one [128, 1536] fp32 psum tile (3 banks,
    double-buffered = 6) -> ONE flat-2D-AP exp per segment (ScalarE,
    score scale folded in) -> Pu cells [128, 32, 512] fp16.
    Flat 2D APs on the exp are load-bearing: 3D/strided APs cost ~2x
    instruction overhead on ACT and also degrade PE issue spacing.
  - AV accumulates [65, 512] fp32 over 16 k-tiles (1 bank per head, 2
    banks per pair; psum total = 6+2 = 8 banks exactly).  Row 64 is the
    denominator: DVE reciprocal_approx_fast + GpSimd partition_broadcast
    + DVE multiply into outT.
  - output projection y^T = wo^T @ outT in fp16 with fp32 psum, emitted
    as 4-matmul groups borrowing a score-psum rotation slot.

Phase-1 projections are folded into the phase-2 unit stream as filler:
only K^T(et0) + Q^T(et0,qc0) precede unit 0 (~12us lead-in), everything
else (V, remaining K^T/Q^T chains, y-groups) fills the PE while ScalarE
drains exp segments.  exp starts ~12us into the kernel instead of ~75us.

PSUM budget: scores 2x[128,3x512] (6 banks) + AV 2x[128,512] (2 banks).
fp16 everywhere 16-bit (same matmul speed as bf16, 8x lower rounding
error on these O(1) tensors); fp32 psum.
"""

import math

import numpy as np

import concourse.bass as bass
import concourse.mybir as mybir
import concourse.tile as tile
from concourse import bacc
from concourse.bass import ds, ts
from concourse.bass_utils import run_bass_kernel_spmd

P = 128
T = 2048        # tokens per batch
DM = 1024       # model dim
E = 512         # per-core projection width (8 heads * 64)
H = 8           # heads per core
HD = 64
CD = DM // P    # contraction chunks over model dim
NKT = T // P    # k tiles per head
QC = 512        # q chunk
NQC = T // QC
TH = T // 2
NP = H // 2     # head pairs
F32 = mybir.dt.float32
FP16 = mybir.dt.float16
EXP = mybir.ActivationFunctionType.Exp


def build(scale: float):
    MDT = FP16
    nc = bacc.Bacc(None, target_bir_lowering=False, debug=False)
    xT = nc.declare_dram_parameter("xT", [DM, T], MDT, isOutput=False)
    wq = nc.declare_dram_parameter("wq", [DM, E], MDT, isOutput=False)
    wk = nc.declare_dram_parameter("wk", [DM, E], MDT, isOutput=False)
    wv = nc.declare_dram_parameter("wv", [DM, E], MDT, isOutput=False)
    wo = nc.declare_dram_parameter("wo", [E, DM], MDT, isOutput=False)
    yT = nc.declare_dram_parameter("yT", [DM, T], MDT, isOutput=True)

    with tile.TileContext(nc) as tc:
        with (
            tc.tile_pool(name="qk", bufs=1) as qkp,
            tc.tile_pool(name="vp", bufs=1) as vp,
            tc.tile_pool(name="misc", bufs=1) as miscp,
            tc.tile_pool(name="wts", bufs=1) as wp,
            tc.tile_pool(name="xt", bufs=1) as xp,
        ):
            QT = qkp.tile([P, 4, T], MDT, tag="QT")
            KT = qkp.tile([P, 4, T], MDT, tag="KT")
            V = vp.tile([P, NKT, H, HD + 1], MDT, tag="V")
            nc.vector.memset(V[:, :, :, HD : HD + 1], 1.0)
            # preload the exp table set so the first real exp doesn't stall
            warm = miscp.tile([1, 8], F32, tag="warm")
            nc.vector.memset(warm[:], 0.0)
            nc.scalar.activation(out=warm[:], in_=warm[:], func=EXP, scale=1.0)

            wq_sb = wp.tile([P, CD, E], MDT, tag="wq")
            wk_sb = wp.tile([P, CD, E], MDT, tag="wk")
            wv_sb = wp.tile([P, CD, E], MDT, tag="wv")
            wq_r = wq.rearrange("(c p) e -> p c e", p=P)
            for c in range(CD):
                nc.sync.dma_start(out=wq_sb[:, c, :], in_=wq_r[:, c, :])
            # prefetch all x tiles (both token halves) up front
            xts = {}
            for th in range(2):
                for c in range(CD):
                    xt = xp.tile([P, TH], MDT, tag=f"x{th}_{c}")
                    nc.sync.dma_start(
                        out=xt[:],
                        in_=xT[c * P : (c + 1) * P, th * TH : (th + 1) * TH],
                    )
                    xts[(th, c)] = xt
            nc.sync.dma_start(out=wk_sb[:], in_=wk.rearrange("(c p) e -> p c e", p=P))
            nc.sync.dma_start(out=wv_sb[:], in_=wv.rearrange("(c p) e -> p c e", p=P))

            with (
                tc.tile_pool(name="outp", bufs=1) as outp,
                tc.tile_pool(name="pbuf", bufs=2) as pbuf,
                tc.tile_pool(name="nrm", bufs=2) as nrmp,
                tc.tile_pool(name="wop", bufs=1) as wop,
                tc.tile_pool(name="yev", bufs=3) as yev,
                tc.tile_pool(name="sps", bufs=2, space="PSUM") as spool,
                tc.tile_pool(name="ovp", bufs=2, space="PSUM") as ovpool,
            ):
                outT = outp.tile([P, 4, T], MDT, tag="outT")
                wo_sb = wop.tile([P, 4, DM], MDT, tag="wo")
                nc.sync.dma_start(
                    out=wo_sb[:], in_=wo.rearrange("(c p) e -> p c e", p=P)
                )

                # ---------- filler chain emitters (each: one 8/4-MM chain) ----
                # Filler psum borrows a score-pool rotation slot ("S" tag) so
                # the total psum footprint stays at 6+2 = 8 banks.
                def kt_chain(et, th, tcl):
                    ps = spool.tile([P, 3 * QC], F32, tag="S")
                    for c in range(CD):
                        nc.tensor.matmul(
                            ps[:, 0:QC],
                            wk_sb[:, c, ts(et, P)],
                            xts[(th, c)][:, ts(tcl, QC)],
                            start=(c == 0),
                            stop=(c == CD - 1),
                        )
                    nc.vector.tensor_copy(
                        KT[:, et, ds(th * TH + tcl * QC, QC)], ps[:, 0:QC]
                    )

                def qt_chain(et, th, tcl):
                    ps = spool.tile([P, 3 * QC], F32, tag="S")
                    for c in range(CD):
                        nc.tensor.matmul(
                            ps[:, 0:QC],
                            wq_sb[:, c, ts(et, P)],
                            xts[(th, c)][:, ts(tcl, QC)],
                            start=(c == 0),
                            stop=(c == CD - 1),
                        )
                    nc.vector.tensor_copy(
                        QT[:, et, ds(th * TH + tcl * QC, QC)], ps[:, 0:QC]
                    )

                def v_chain(tt):
                    th, ttl = tt // (TH // P), tt % (TH // P)
                    ps = spool.tile([P, 3 * QC], F32, tag="S")
                    for c in range(CD):
                        nc.tensor.matmul(
                            ps[:, 0:E],
                            xts[(th, c)][:, ts(ttl, P)],
                            wv_sb[:, c, :],
                            start=(c == 0),
                            stop=(c == CD - 1),
                        )
                    nc.vector.tensor_copy(
                        V[:, tt, :, 0:HD],
                        ps[:, 0:E].rearrange("p (h e) -> p h e", h=H),
                    )

                def y_group(et, tcq):
                    ps = spool.tile([P, 3 * QC], F32, tag="S")
                    for fc in range(4):
                        nc.tensor.matmul(
                            ps[:, 0:QC],
                            wo_sb[:, fc, ts(et, P)],
                            outT[:, fc, ts(tcq, QC)],
                            start=(fc == 0),
                            stop=(fc == 3),
                        )
                    yt = yev.tile([P, QC], MDT, tag="ye")
                    nc.vector.tensor_copy(yt[:], ps[:, 0:QC])
                    nc.sync.dma_start(
                        out=yT[et * P : (et + 1) * P, ts(tcq, QC)], in_=yt[:]
                    )

                # ---------- phase-2 unit helpers ------------------------------
                # cell = kt*2 + parity; 11 segments of [3,3,3,3,3,3,3,3,3,3,2]
                SEGS = [(s, min(s + 3, 32)) for s in range(0, 32, 3)]

                def emit_score_seg(j, qc, c0, c1, Pu):
                    n = c1 - c0
                    sp = spool.tile([P, 3 * QC], F32, tag="S")
                    for u, cell in enumerate(range(c0, c1)):
                        kt, par = cell >> 1, cell & 1
                        off = par * HD
                        nc.tensor.matmul(
                            sp[:, u * QC : (u + 1) * QC],
                            KT[off : off + HD, j, kt * P : (kt + 1) * P],
                            QT[off : off + HD, j, ts(qc, QC)],
                            start=True,
                            stop=True,
                        )
                    nc.scalar.activation(
                        out=Pu[:, c0:c1, :].rearrange("p a b -> p (a b)"),
                        in_=sp[:, 0 : n * QC],
                        func=EXP,
                        scale=scale,
                    )

                def emit_av(j, par, opsum, Pu, kt):
                    h = 2 * j + par
                    nc.tensor.matmul(
                        opsum[0 : HD + 1, :],
                        V[:, kt, h, :],
                        Pu[:, 2 * kt + par, :],
                        start=(kt == 0),
                        stop=(kt == NKT - 1),
                    )

                def emit_finish(opsum, j, par, qc):
                    off = par * HD
                    den = nrmp.tile([1, QC], F32, tag="dn")
                    nc.vector.tensor_copy(den[:], opsum[HD : HD + 1, :])
                    recip = nrmp.tile([1, QC], F32, tag="rc")
                    nc.vector.reciprocal_approx_fast(recip[:], den[:])
                    bcs = nrmp.tile([HD, QC], F32, tag="bcs")
                    nc.gpsimd.partition_broadcast(bcs[:], recip[:], channels=HD)
                    nc.vector.tensor_mul(
                        outT[off : off + HD, j, ts(qc, QC)],
                        opsum[0:HD, :],
                        bcs[:],
                    )

                # ---------- static filler schedule ----------------------------
                # unit u = (qc, j) with qc outer: u = qc*4 + j.
                # Constraints encoded here: KT(et_j)/QT(et_j, qc) chains are
                # emitted at least one unit before the unit that consumes
                # them; V th0 before unit 1's early AV, V th1 within unit 1;
                # y(tcq) only after all 4 pairs of that tcq have finished
                # (AV of (tcq, j3) runs in the next unit).
                FILLER = {
                    0: [("kt", 1, 0, 0), ("kt", 1, 0, 1), ("kt", 1, 1, 0),
                        ("kt", 1, 1, 1), ("qt", 1, 0, 0),
                        ("v", 0), ("v", 1), ("v", 2), ("v", 3),
                        ("v", 4), ("v", 5), ("v", 6), ("v", 7)],
                    1: [("v", 8), ("v", 9), ("v", 10), ("v", 11),
                        ("v", 12), ("v", 13), ("v", 14), ("v", 15),
                        ("kt", 2, 0, 0), ("kt", 2, 0, 1), ("kt", 2, 1, 0),
                        ("kt", 2, 1, 1), ("qt", 2, 0, 0)],
                    2: [("kt", 3, 0, 0), ("kt", 3, 0, 1), ("kt", 3, 1, 0),
                        ("kt", 3, 1, 1), ("qt", 3, 0, 0), ("qt", 0, 0, 1)],
                    3: [("qt", 1, 0, 1), ("qt", 2, 0, 1), ("qt", 3, 0, 1)],
                    4: [("qt", 0, 1, 0), ("qt", 1, 1, 0)],
                    5: [("qt", 2, 1, 0), ("qt", 3, 1, 0),
                        ("y", 0, 0), ("y", 1, 0)],
                    6: [("qt", 0, 1, 1), ("qt", 1, 1, 1),
                        ("y", 2, 0), ("y", 3, 0), ("y", 4, 0)],
                    7: [("qt", 2, 1, 1), ("qt", 3, 1, 1),
                        ("y", 5, 0), ("y", 6, 0), ("y", 7, 0)],
                    8: [],
                    9: [("y", 0, 1), ("y", 1, 1), ("y", 2, 1)],
                    10: [("y", 3, 1), ("y", 4, 1), ("y", 5, 1)],
                    11: [("y", 6, 1), ("y", 7, 1)],
                    12: [],
                    13: [("y", 0, 2), ("y", 1, 2), ("y", 2, 2)],
                    14: [("y", 3, 2), ("y", 4, 2), ("y", 5, 2)],
                    15: [("y", 6, 2), ("y", 7, 2)],
                }

                def emit_filler(item):
                    kind = item[0]
                    if kind == "kt":
                        kt_chain(*item[1:])
                    elif kind == "qt":
                        qt_chain(*item[1:])
                    elif kind == "v":
                        v_chain(item[1])
                    else:
                        y_group(item[1], item[2])

                # ---------- lead-in: only what unit 0 needs -------------------
                kt_chain(0, 0, 0)
                kt_chain(0, 0, 1)
                kt_chain(0, 1, 0)
                kt_chain(0, 1, 1)
                qt_chain(0, 0, 0)

                # ---------- the unit loop -------------------------------------
                units = [(qc, j) for qc in range(NQC) for j in range(NP)]
                # AV-MM spread across the 11 segments (32 per unit)
                SPREAD = [3, 3, 3, 3, 3, 3, 3, 3, 3, 3, 2]
                # unit 1 consumes V-th1 chains first, AV back-loaded
                SPREAD_U1 = [0, 0, 2, 2, 4, 4, 4, 4, 4, 4, 0]

                prev = None  # (opsum_e, opsum_o, Pu, j, qc)
                for ui, (qc, j) in enumerate(units):
                    Pu = pbuf.tile([P, 2 * NKT, QC], MDT, tag="P")
                    filler = list(FILLER.get(ui, []))
                    # split filler across segments roughly evenly
                    nf = len(filler)
                    spread = SPREAD_U1 if ui == 1 else SPREAD
                    av_i = 0
                    for si, (c0, c1) in enumerate(SEGS):
                        emit_score_seg(j, qc, c0, c1, Pu)
                        if prev is not None:
                            pe, po, pPu, pj, pqc = prev
                            for _ in range(spread[si]):
                                if av_i >= 2 * NKT:
                                    break
                                kt, par = av_i >> 1, av_i & 1
                                emit_av(pj, par, pe if par == 0 else po,
                                        pPu, kt)
                                av_i += 1
                        # one filler chain after each segment, round-robin
                        want = (nf * (si + 1)) // len(SEGS)
                        while len(filler) > nf - want and filler:
                            emit_filler(filler.pop(0))
                    if prev is not None:
                        pe, po, pPu, pj, pqc = prev
                        while av_i < 2 * NKT:
                            kt, par = av_i >> 1, av_i & 1
                            emit_av(pj, par, pe if par == 0 else po, pPu, kt)
                            av_i += 1
                        emit_finish(pe, pj, 0, pqc)
                        emit_finish(po, pj, 1, pqc)
                    opsum_e = ovpool.tile([P, QC], F32, tag="ov")
                    opsum_o = ovpool.tile([P, QC], F32, tag="ov")
                    prev = (opsum_e, opsum_o, Pu, j, qc)

                # ---------- tail ----------------------------------------------
                pe, po, pPu, pj, pqc = prev
                for kt in range(NKT):
                    emit_av(pj, 0, pe, pPu, kt)
                    emit_av(pj, 1, po, pPu, kt)
                emit_finish(pe, pj, 0, pqc)
                emit_finish(po, pj, 1, pqc)
                for et in range(DM // P):
                    y_group(et, 3)

    nc.compile()
    return nc


_CACHE: dict = {}


def _get_program(scale: float):
    key = round(float(scale), 12)
    if key not in _CACHE:
        _CACHE[key] = build(key)
    return _CACHE[key]


def _make_in_maps(x, w_qkv, w_out):
    cdt = np.float16
    xTs = [np.ascontiguousarray(x[b].T).astype(cdt) for b in range(4)]
    wslices = []
    for hg in range(2):
        sl = slice(hg * E, (hg + 1) * E)
        wslices.append(
            {
                "wq": np.ascontiguousarray(w_qkv[0 * DM :][sl, :].T).astype(cdt),
                "wk": np.ascontiguousarray(w_qkv[1 * DM :][sl, :].T).astype(cdt),
                "wv": np.ascontiguousarray(w_qkv[2 * DM :][sl, :].T).astype(cdt),
                "wo": np.ascontiguousarray(w_out[:, sl].T).astype(cdt),
            }
        )
    in_maps = []
    for c in range(8):
        b, hg = c // 2, c % 2
        m = {"xT": xTs[b]}
        m.update(wslices[hg])
        in_maps.append(m)
    return in_maps


def _execute(x, w_qkv, w_out, rescale, **spmd_kwargs):
    scale = float(np.asarray(rescale)) / math.sqrt(HD)
    nc = _get_program(scale)
    in_maps = _make_in_maps(x, w_qkv, w_out)
    return run_bass_kernel_spmd(nc, in_maps, list(range(8)), **spmd_kwargs)


def kernel(x, w_qkv, w_out, b_out, rescale):
    x = np.asarray(x, dtype=np.float32)
    w_qkv = np.asarray(w_qkv, dtype=np.float32)
    w_out = np.asarray(w_out, dtype=np.float32)
    b_out = np.asarray(b_out, dtype=np.float32)
    res = _execute(x, w_qkv, w_out, rescale).results
    y = np.empty((4, T, DM), dtype=np.float32)
    for b in range(4):
        acc = res[2 * b]["yT"].astype(np.float32) + res[2 * b + 1]["yT"].astype(
            np.float32
        )
        y[b] = acc.T + b_out
    return y


# revision 14
# speedup vs baseline: 1.0825x; 1.0825x over previous
"""DriftAwareMultiHeadAttention on 8 Trainium2 NeuronCores.

Sharding (per spec hint): core c -> (batch b = c//2, head-group hg = c%2).
Each core runs the QKV projection column-parallel over its 8 heads, full
attention for those heads, and a row-parallel partial output projection.
Host gather: y[b] = (yT[2b] + yT[2b+1]).T + b_out.

v2: paired-head phase 2.  The per-core 8 heads are processed as 4 pairs
(2j, 2j+1); the pair's K^T/Q^T live in the same e-tile at partition
offsets 0/64, so their K=64 score matmuls carry tile_position (0,0) and
(64,0) and run CONCURRENTLY on disjoint row-groups of the PE array
(2 matmuls per ~N cycles instead of 1).  This removes the half-array
waste of the hd=64 contraction: scores cost ~56us instead of ~110us.

Device layout is feature-on-partition / token-on-free throughout:
  - Q^T, K^T: [512, 2048] fp16 (pair j -> e-tile j, head parity ->
    partition offset 0/64).  V: [tokens, 8 heads x (64+1)] fp16 -- the
    extra "ones" column makes the AV matmul emit the softmax denominator
    in psum row 64 for free.
  - Unit (qc, j): 32 cells (cell = kt*2 + parity) of S^T score matmuls,
    emitted in cell order so adjacent matmuls alternate row-groups.
    Segments of 3 cells -> one [128, 1536] fp32 psum tile (3 banks,
    double-buffered = 6) -> ONE flat-2D-AP exp per segment (ScalarE,
    score scale folded in) -> Pu cells [128, 32, 512] fp16.
    Flat 2D APs on the exp are load-bearing: 3D/strided APs cost ~2x
    instruction overhead on ACT and also degrade PE issue spacing.
  - AV accumulates [65, 512] fp32 over 16 k-tiles (1 bank per head, 2
    banks per pair; psum total = 6+2 = 8 banks exactly).  Row 64 is the
    denominator: DVE reciprocal_approx_fast + GpSimd partition_broadcast
    + DVE multiply into outT.
  - output projection y^T = wo^T @ outT in fp16 with fp32 psum, emitted
    as 4-matmul groups borrowing a score-psum rotation slot.

Phase-1 projections are folded into the phase-2 unit stream as filler:
only K^T(et0) + Q^T(et0,qc0) precede unit 0 (~12us lead-in), everything
else (V, remaining K^T/Q^T chains, y-groups) fills the PE while ScalarE
drains exp segments.  exp starts ~12us into the kernel instead of ~75us.

PSUM budget: scores 2x[128,3x512] (6 banks) + AV 2x[128,512] (2 banks).
fp16 everywhere 16-bit (same matmul speed as bf16, 8x lower rounding
error on these O(1) tensors); fp32 psum.
"""

import math

import numpy as np

import concourse.bass as bass
import concourse.mybir as mybir
import concourse.tile as tile
from concourse import bacc
from concourse.bass import ds, ts
from concourse.bass_utils import run_bass_kernel_spmd

P = 128
T = 2048        # tokens per batch
DM = 1024       # model dim
E = 512         # per-core projection width (8 heads * 64)
H = 8           # heads per core
HD = 64
CD = DM // P    # contraction chunks over model dim
NKT = T // P    # k tiles per head
QC = 512        # q chunk
NQC = T // QC
TH = T // 2
NP = H // 2     # head pairs
F32 = mybir.dt.float32
FP16 = mybir.dt.float16
EXP = mybir.ActivationFunctionType.Exp


def build(scale: float):
    MDT = FP16
    nc = bacc.Bacc(None, target_bir_lowering=False, debug=False)
    xT = nc.declare_dram_parameter("xT", [DM, T], MDT, isOutput=False)
    wq = nc.declare_dram_parameter("wq", [DM, E], MDT, isOutput=False)
    wk = nc.declare_dram_parameter("wk", [DM, E], MDT, isOutput=False)
    wv = nc.declare_dram_parameter("wv", [DM, E], MDT, isOutput=False)
    wo = nc.declare_dram_parameter("wo", [E, DM], MDT, isOutput=False)
    yT = nc.declare_dram_parameter("yT", [DM, T], MDT, isOutput=True)

    with tile.TileContext(nc) as tc:
        with (
            tc.tile_pool(name="qk", bufs=1) as qkp,
            tc.tile_pool(name="vp", bufs=1) as vp,
            tc.tile_pool(name="misc", bufs=1) as miscp,
            tc.tile_pool(name="wts", bufs=1) as wp,
            tc.tile_pool(name="xt", bufs=1) as xp,
        ):
            QT = qkp.tile([P, 4, T], MDT, tag="QT")
            KT = qkp.tile([P, 4, T], MDT, tag="KT")
            V = vp.tile([P, NKT, H, HD + 1], MDT, tag="V")
            nc.vector.memset(V[:, :, :, HD : HD + 1], 1.0)
            # preload the exp table set so the first real exp doesn't stall
            warm = miscp.tile([1, 8], F32, tag="warm")
            nc.vector.memset(warm[:], 0.0)
            nc.scalar.activation(out=warm[:], in_=warm[:], func=EXP, scale=1.0)

            wq_sb = wp.tile([P, CD, E], MDT, tag="wq")
            wk_sb = wp.tile([P, CD, E], MDT, tag="wk")
            wv_sb = wp.tile([P, CD, E], MDT, tag="wv")
            # DMA order is the critical path to the first exp: the lead-in
            # KT(et0) chains need wk + x(th0,th1); issue those first.
            wk_r = wk.rearrange("(c p) e -> p c e", p=P)
            for c in range(CD):
                nc.sync.dma_start(out=wk_sb[:, c, :], in_=wk_r[:, c, :])
            xts = {}
            for th in range(2):
                for c in range(CD):
                    xt = xp.tile([P, TH], MDT, tag=f"x{th}_{c}")
                    nc.sync.dma_start(
                        out=xt[:],
                        in_=xT[c * P : (c + 1) * P, th * TH : (th + 1) * TH],
                    )
                    xts[(th, c)] = xt
            wq_r = wq.rearrange("(c p) e -> p c e", p=P)
            for c in range(CD):
                nc.sync.dma_start(out=wq_sb[:, c, :], in_=wq_r[:, c, :])
            nc.sync.dma_start(out=wv_sb[:], in_=wv.rearrange("(c p) e -> p c e", p=P))

            with (
                tc.tile_pool(name="outp", bufs=1) as outp,
                tc.tile_pool(name="pbuf", bufs=2) as pbuf,
                tc.tile_pool(name="nrm", bufs=2) as nrmp,
                tc.tile_pool(name="wop", bufs=1) as wop,
                tc.tile_pool(name="yev", bufs=3) as yev,
                tc.tile_pool(name="sps", bufs=2, space="PSUM") as spool,
                tc.tile_pool(name="ovp", bufs=2, space="PSUM") as ovpool,
                tc.tile_pool(name="p1", bufs=2, space="PSUM") as p1pool,
            ):
                outT = outp.tile([P, 4, T], MDT, tag="outT")
                wo_sb = wop.tile([P, 4, DM], MDT, tag="wo")
                nc.sync.dma_start(
                    out=wo_sb[:], in_=wo.rearrange("(c p) e -> p c e", p=P)
                )

                # ---------- filler chain emitters (each: one 8/4-MM chain) ----
                # Filler runs in its own double-buffered 2-bank psum pool so
                # it never perturbs the score-segment double-buffering.
                # Total psum: scores 2x2 + filler 2 + AV 2 = 8 banks.
                def kt_chain(et, th, tcl):
                    ps = p1pool.tile([P, QC], F32, tag="pp")
                    for c in range(CD):
                        nc.tensor.matmul(
                            ps[:],
                            wk_sb[:, c, ts(et, P)],
                            xts[(th, c)][:, ts(tcl, QC)],
                            start=(c == 0),
                            stop=(c == CD - 1),
                        )
                    nc.vector.tensor_copy(
                        KT[:, et, ds(th * TH + tcl * QC, QC)], ps[:]
                    )

                def qt_chain(et, th, tcl):
                    ps = p1pool.tile([P, QC], F32, tag="pp")
                    for c in range(CD):
                        nc.tensor.matmul(
                            ps[:],
                            wq_sb[:, c, ts(et, P)],
                            xts[(th, c)][:, ts(tcl, QC)],
                            start=(c == 0),
                            stop=(c == CD - 1),
                        )
                    nc.vector.tensor_copy(
                        QT[:, et, ds(th * TH + tcl * QC, QC)], ps[:]
                    )

                def v_chain(tt):
                    th, ttl = tt // (TH // P), tt % (TH // P)
                    ps = p1pool.tile([P, E], F32, tag="pp")
                    for c in range(CD):
                        nc.tensor.matmul(
                            ps[:],
                            xts[(th, c)][:, ts(ttl, P)],
                            wv_sb[:, c, :],
                            start=(c == 0),
                            stop=(c == CD - 1),
                        )
                    nc.vector.tensor_copy(
                        V[:, tt, :, 0:HD],
                        ps[:].rearrange("p (h e) -> p h e", h=H),
                    )

                def y_group(et, tcq):
                    ps = p1pool.tile([P, QC], F32, tag="pp")
                    for fc in range(4):
                        nc.tensor.matmul(
                            ps[:],
                            wo_sb[:, fc, ts(et, P)],
                            outT[:, fc, ts(tcq, QC)],
                            start=(fc == 0),
                            stop=(fc == 3),
                        )
                    yt = yev.tile([P, QC], MDT, tag="ye")
                    nc.vector.tensor_copy(yt[:], ps[:])
                    nc.sync.dma_start(
                        out=yT[et * P : (et + 1) * P, ts(tcq, QC)], in_=yt[:]
                    )

                # ---------- phase-2 unit helpers ------------------------------
                # cell = kt*2 + parity; 16 segments of 2 cells
                SEGS = [(s, s + 2) for s in range(0, 32, 2)]

                def emit_score_seg(j, qc, c0, c1, Pu):
                    n = c1 - c0
                    sp = spool.tile([P, 2 * QC], F32, tag="S")
                    for u, cell in enumerate(range(c0, c1)):
                        kt, par = cell >> 1, cell & 1
                        off = par * HD
                        nc.tensor.matmul(
                            sp[:, u * QC : (u + 1) * QC],
                            KT[off : off + HD, j, kt * P : (kt + 1) * P],
                            QT[off : off + HD, j, ts(qc, QC)],
                            start=True,
                            stop=True,
                        )
                    nc.scalar.activation(
                        out=Pu[:, c0:c1, :].rearrange("p a b -> p (a b)"),
                        in_=sp[:, 0 : n * QC],
                        func=EXP,
                        scale=scale,
                    )

                def emit_av(j, par, opsum, Pu, kt):
                    h = 2 * j + par
                    nc.tensor.matmul(
                        opsum[0 : HD + 1, :],
                        V[:, kt, h, :],
                        Pu[:, 2 * kt + par, :],
                        start=(kt == 0),
                        stop=(kt == NKT - 1),
                    )

                def emit_finish(opsum, j, par, qc):
                    off = par * HD
                    den = nrmp.tile([1, QC], F32, tag="dn")
                    nc.vector.tensor_copy(den[:], opsum[HD : HD + 1, :])
                    recip = nrmp.tile([1, QC], F32, tag="rc")
                    nc.vector.reciprocal_approx_fast(recip[:], den[:])
                    bcs = nrmp.tile([HD, QC], F32, tag="bcs")
                    nc.gpsimd.partition_broadcast(bcs[:], recip[:], channels=HD)
                    nc.vector.tensor_mul(
                        outT[off : off + HD, j, ts(qc, QC)],
                        opsum[0:HD, :],
                        bcs[:],
                    )

                # ---------- static filler schedule ----------------------------
                # unit u = (qc, j) with qc outer: u = qc*4 + j.
                # Constraints encoded here: KT(et_j)/QT(et_j, qc) chains are
                # emitted at least one unit before the unit that consumes
                # them; V th0 before unit 1's early AV, V th1 within unit 1;
                # y(tcq) only after all 4 pairs of that tcq have finished
                # (AV of (tcq, j3) runs in the next unit).
                FILLER = {
                    0: [("kt", 1, 0, 0), ("kt", 1, 0, 1), ("kt", 1, 1, 0),
                        ("kt", 1, 1, 1), ("qt", 1, 0, 0),
                        ("v", 0), ("v", 1), ("v", 2), ("v", 3),
                        ("v", 4), ("v", 5), ("v", 6), ("v", 7)],
                    1: [("v", 8), ("v", 9), ("v", 10), ("v", 11),
                        ("v", 12), ("v", 13), ("v", 14), ("v", 15),
                        ("kt", 2, 0, 0), ("kt", 2, 0, 1), ("kt", 2, 1, 0),
                        ("kt", 2, 1, 1), ("qt", 2, 0, 0)],
                    2: [("kt", 3, 0, 0), ("kt", 3, 0, 1), ("kt", 3, 1, 0),
                        ("kt", 3, 1, 1), ("qt", 3, 0, 0), ("qt", 0, 0, 1)],
                    3: [("qt", 1, 0, 1), ("qt", 2, 0, 1), ("qt", 3, 0, 1)],
                    4: [("qt", 0, 1, 0), ("qt", 1, 1, 0)],
                    5: [("qt", 2, 1, 0), ("qt", 3, 1, 0),
                        ("y", 0, 0), ("y", 1, 0)],
                    6: [("qt", 0, 1, 1), ("qt", 1, 1, 1),
                        ("y", 2, 0), ("y", 3, 0), ("y", 4, 0)],
                    7: [("qt", 2, 1, 1), ("qt", 3, 1, 1),
                        ("y", 5, 0), ("y", 6, 0), ("y", 7, 0)],
                    8: [],
                    9: [("y", 0, 1), ("y", 1, 1), ("y", 2, 1)],
                    10: [("y", 3, 1), ("y", 4, 1), ("y", 5, 1)],
                    11: [("y", 6, 1), ("y", 7, 1)],
                    12: [],
                    13: [("y", 0, 2), ("y", 1, 2), ("y", 2, 2)],
                    14: [("y", 3, 2), ("y", 4, 2), ("y", 5, 2)],
                    15: [("y", 6, 2), ("y", 7, 2)],
                }

                def emit_filler(item):
                    kind = item[0]
                    if kind == "kt":
                        kt_chain(*item[1:])
                    elif kind == "qt":
                        qt_chain(*item[1:])
                    elif kind == "v":
                        v_chain(item[1])
                    else:
                        y_group(item[1], item[2])

                # ---------- lead-in: only what unit 0 needs -------------------
                kt_chain(0, 0, 0)
                kt_chain(0, 0, 1)
                kt_chain(0, 1, 0)
                kt_chain(0, 1, 1)
                qt_chain(0, 0, 0)

                # ---------- the unit loop -------------------------------------
                units = [(qc, j) for qc in range(NQC) for j in range(NP)]
                # AV-MM spread across the 16 segments (32 per unit)
                SPREAD = [2] * 16
                # unit 1 consumes V-th1 chains first, AV back-loaded
                SPREAD_U1 = [0, 0, 0, 0, 3, 3, 3, 3, 3, 3, 3, 3, 2, 2, 2, 0]

                prev = None  # (opsum_e, opsum_o, Pu, j, qc)
                for ui, (qc, j) in enumerate(units):
                    Pu = pbuf.tile([P, 2 * NKT, QC], MDT, tag="P")
                    filler = list(FILLER.get(ui, []))
                    # split filler across segments roughly evenly
                    nf = len(filler)
                    spread = SPREAD_U1 if ui == 1 else SPREAD
                    av_i = 0
                    for si, (c0, c1) in enumerate(SEGS):
                        emit_score_seg(j, qc, c0, c1, Pu)
                        if prev is not None:
                            pe, po, pPu, pj, pqc = prev
                            for _ in range(spread[si]):
                                if av_i >= 2 * NKT:
                                    break
                                kt, par = av_i >> 1, av_i & 1
                                emit_av(pj, par, pe if par == 0 else po,
                                        pPu, kt)
                                av_i += 1
                        # one filler chain after each segment, round-robin
                        want = (nf * (si + 1)) // len(SEGS)
                        while len(filler) > nf - want and filler:
                            emit_filler(filler.pop(0))
                    if prev is not None:
                        pe, po, pPu, pj, pqc = prev
                        while av_i < 2 * NKT:
                            kt, par = av_i >> 1, av_i & 1
                            emit_av(pj, par, pe if par == 0 else po, pPu, kt)
                            av_i += 1
                        emit_finish(pe, pj, 0, pqc)
                        emit_finish(po, pj, 1, pqc)
                    opsum_e = ovpool.tile([P, QC], F32, tag="ov")
                    opsum_o = ovpool.tile([P, QC], F32, tag="ov")
                    prev = (opsum_e, opsum_o, Pu, j, qc)

                # ---------- tail ----------------------------------------------
                pe, po, pPu, pj, pqc = prev
                for kt in range(NKT):
                    emit_av(pj, 0, pe, pPu, kt)
                    emit_av(pj, 1, po, pPu, kt)
                emit_finish(pe, pj, 0, pqc)
                emit_finish(po, pj, 1, pqc)
                for et in range(DM // P):
                    y_group(et, 3)

    nc.compile()
    return nc


_CACHE: dict = {}


def _get_program(scale: float):
    key = round(float(scale), 12)
    if key not in _CACHE:
        _CACHE[key] = build(key)
    return _CACHE[key]


def _make_in_maps(x, w_qkv, w_out):
    cdt = np.float16
    xTs = [np.ascontiguousarray(x[b].T).astype(cdt) for b in range(4)]
    wslices = []
    for hg in range(2):
        sl = slice(hg * E, (hg + 1) * E)
        wslices.append(
            {
                "wq": np.ascontiguousarray(w_qkv[0 * DM :][sl, :].T).astype(cdt),
                "wk": np.ascontiguousarray(w_qkv[1 * DM :][sl, :].T).astype(cdt),
                "wv": np.ascontiguousarray(w_qkv[2 * DM :][sl, :].T).astype(cdt),
                "wo": np.ascontiguousarray(w_out[:, sl].T).astype(cdt),
            }
        )
    in_maps = []
    for c in range(8):
        b, hg = c // 2, c % 2
        m = {"xT": xTs[b]}
        m.update(wslices[hg])
        in_maps.append(m)
    return in_maps


def _execute(x, w_qkv, w_out, rescale, **spmd_kwargs):
    scale = float(np.asarray(rescale)) / math.sqrt(HD)
    nc = _get_program(scale)
    in_maps = _make_in_maps(x, w_qkv, w_out)
    return run_bass_kernel_spmd(nc, in_maps, list(range(8)), **spmd_kwargs)


def kernel(x, w_qkv, w_out, b_out, rescale):
    x = np.asarray(x, dtype=np.float32)
    w_qkv = np.asarray(w_qkv, dtype=np.float32)
    w_out = np.asarray(w_out, dtype=np.float32)
    b_out = np.asarray(b_out, dtype=np.float32)
    res = _execute(x, w_qkv, w_out, rescale).results
    y = np.empty((4, T, DM), dtype=np.float32)
    for b in range(4):
        acc = res[2 * b]["yT"].astype(np.float32) + res[2 * b + 1]["yT"].astype(
            np.float32
        )
        y[b] = acc.T + b_out
    return y


# revision 19
# speedup vs baseline: 1.0971x; 1.0135x over previous
"""DriftAwareMultiHeadAttention on 8 Trainium2 NeuronCores.

Sharding (per spec hint): core c -> (batch b = c//2, head-group hg = c%2).
Each core runs the QKV projection column-parallel over its 8 heads, full
attention for those heads, and a row-parallel partial output projection.
Host gather: y[b] = (yT[2b] + yT[2b+1]).T + b_out.

v2: paired-head phase 2.  The per-core 8 heads are processed as 4 pairs
(2j, 2j+1); the pair's K^T/Q^T live in the same e-tile at partition
offsets 0/64, so their K=64 score matmuls carry tile_position (0,0) and
(64,0) and run CONCURRENTLY on disjoint row-groups of the PE array
(2 matmuls per ~N cycles instead of 1).  This removes the half-array
waste of the hd=64 contraction: scores cost ~56us instead of ~110us.

Device layout is feature-on-partition / token-on-free throughout:
  - Q^T, K^T: [512, 2048] fp16 (pair j -> e-tile j, head parity ->
    partition offset 0/64).  V: [tokens, 8 heads x (64+1)] fp16 -- the
    extra "ones" column makes the AV matmul emit the softmax denominator
    in psum row 64 for free.
  - Unit (qc, j): 32 cells (cell = kt*2 + parity) of S^T score matmuls,
    emitted in cell order so adjacent matmuls alternate row-groups.
    Segments of 3 cells -> one [128, 1536] fp32 psum tile (3 banks,
    double-buffered = 6) -> ONE flat-2D-AP exp per segment (ScalarE,
    score scale folded in) -> Pu cells [128, 32, 512] fp16.
    Flat 2D APs on the exp are load-bearing: 3D/strided APs cost ~2x
    instruction overhead on ACT and also degrade PE issue spacing.
  - AV accumulates [65, 512] fp32 over 16 k-tiles (1 bank per head, 2
    banks per pair; psum total = 6+2 = 8 banks exactly).  Row 64 is the
    denominator: DVE reciprocal_approx_fast + GpSimd partition_broadcast
    + DVE multiply into outT.
  - output projection y^T = wo^T @ outT in fp16 with fp32 psum, emitted
    as 4-matmul groups borrowing a score-psum rotation slot.

Phase-1 projections are folded into the phase-2 unit stream as filler:
only K^T(et0) + Q^T(et0,qc0) precede unit 0 (~12us lead-in), everything
else (V, remaining K^T/Q^T chains, y-groups) fills the PE while ScalarE
drains exp segments.  exp starts ~12us into the kernel instead of ~75us.

PSUM budget: scores 2x[128,3x512] (6 banks) + AV 2x[128,512] (2 banks).
fp16 everywhere 16-bit (same matmul speed as bf16, 8x lower rounding
error on these O(1) tensors); fp32 psum.
"""

import math

import numpy as np

import concourse.bass as bass
import concourse.mybir as mybir
import concourse.tile as tile
from concourse import bacc
from concourse.bass import ds, ts
from concourse.bass_utils import run_bass_kernel_spmd

P = 128
T = 2048        # tokens per batch
DM = 1024       # model dim
E = 512         # per-core projection width (8 heads * 64)
H = 8           # heads per core
HD = 64
CD = DM // P    # contraction chunks over model dim
NKT = T // P    # k tiles per head
QC = 512        # q chunk
NQC = T // QC
TH = T // 2
NP = H // 2     # head pairs
F32 = mybir.dt.float32
FP16 = mybir.dt.float16
EXP = mybir.ActivationFunctionType.Exp


def build(scale: float):
    MDT = FP16
    nc = bacc.Bacc(None, target_bir_lowering=False, debug=False)
    xT = nc.declare_dram_parameter("xT", [DM, T], MDT, isOutput=False)
    wq = nc.declare_dram_parameter("wq", [DM, E], MDT, isOutput=False)
    wk = nc.declare_dram_parameter("wk", [DM, E], MDT, isOutput=False)
    wv = nc.declare_dram_parameter("wv", [DM, E], MDT, isOutput=False)
    wo = nc.declare_dram_parameter("wo", [E, DM], MDT, isOutput=False)
    yT = nc.declare_dram_parameter("yT", [DM, T], MDT, isOutput=True)

    with tile.TileContext(nc) as tc:
        with (
            tc.tile_pool(name="qk", bufs=1) as qkp,
            tc.tile_pool(name="vp", bufs=1) as vp,
            tc.tile_pool(name="misc", bufs=1) as miscp,
            tc.tile_pool(name="wts", bufs=1) as wp,
            tc.tile_pool(name="xt", bufs=1) as xp,
        ):
            QT = qkp.tile([P, 4, T], MDT, tag="QT")
            KT = qkp.tile([P, 4, T], MDT, tag="KT")
            V = vp.tile([P, NKT, H, HD + 1], MDT, tag="V")
            nc.vector.memset(V[:, :, :, HD : HD + 1], 1.0)
            # preload the exp table set so the first real exp doesn't stall
            warm = miscp.tile([1, 8], F32, tag="warm")
            nc.vector.memset(warm[:], 0.0)
            nc.scalar.activation(out=warm[:], in_=warm[:], func=EXP, scale=1.0)

            wq_sb = wp.tile([P, CD, E], MDT, tag="wq")
            wk_sb = wp.tile([P, CD, E], MDT, tag="wk")
            wv_sb = wp.tile([P, CD, E], MDT, tag="wv")
            # DMA order is the critical path to the first exp: the first
            # unit's cells 0..15 touch only th0 tokens, so KT(et0,th0) +
            # QT(et0,th0,tcl0) gate it -> wk, x(th0), wq first; x(th1), wv
            # after.
            xts = {}

            def _dma_x(th):
                for c in range(CD):
                    xt = xp.tile([P, TH], MDT, tag=f"x{th}_{c}")
                    nc.sync.dma_start(
                        out=xt[:],
                        in_=xT[c * P : (c + 1) * P, th * TH : (th + 1) * TH],
                    )
                    xts[(th, c)] = xt

            wk_r = wk.rearrange("(c p) e -> p c e", p=P)
            for c in range(CD):
                nc.sync.dma_start(out=wk_sb[:, c, :], in_=wk_r[:, c, :])
            _dma_x(0)
            wq_r = wq.rearrange("(c p) e -> p c e", p=P)
            for c in range(CD):
                nc.sync.dma_start(out=wq_sb[:, c, :], in_=wq_r[:, c, :])
            _dma_x(1)
            nc.sync.dma_start(out=wv_sb[:], in_=wv.rearrange("(c p) e -> p c e", p=P))

            with (
                tc.tile_pool(name="outp", bufs=1) as outp,
                tc.tile_pool(name="pbuf", bufs=2) as pbuf,
                tc.tile_pool(name="nrm", bufs=2) as nrmp,
                tc.tile_pool(name="wop", bufs=1) as wop,
                tc.tile_pool(name="yev", bufs=3) as yev,
                tc.tile_pool(name="sps", bufs=2, space="PSUM") as spool,
                tc.tile_pool(name="ovp", bufs=2, space="PSUM") as ovpool,
                tc.tile_pool(name="p1", bufs=2, space="PSUM") as p1pool,
            ):
                outT = outp.tile([P, 4, T], MDT, tag="outT")
                wo_sb = wop.tile([P, 4, DM], MDT, tag="wo")
                nc.sync.dma_start(
                    out=wo_sb[:], in_=wo.rearrange("(c p) e -> p c e", p=P)
                )

                # ---------- filler chain emitters (each: one 8/4-MM chain) ----
                # Filler runs in its own double-buffered 2-bank psum pool so
                # it never perturbs the score-segment double-buffering.
                # Total psum: scores 2x2 + filler 2 + AV 2 = 8 banks.
                def kt_chain(et, th, tcl):
                    ps = p1pool.tile([P, QC], F32, tag="pp")
                    for c in range(CD):
                        nc.tensor.matmul(
                            ps[:],
                            wk_sb[:, c, ts(et, P)],
                            xts[(th, c)][:, ts(tcl, QC)],
                            start=(c == 0),
                            stop=(c == CD - 1),
                        )
                    nc.vector.tensor_copy(
                        KT[:, et, ds(th * TH + tcl * QC, QC)], ps[:]
                    )

                def qt_chain(et, th, tcl):
                    ps = p1pool.tile([P, QC], F32, tag="pp")
                    for c in range(CD):
                        nc.tensor.matmul(
                            ps[:],
                            wq_sb[:, c, ts(et, P)],
                            xts[(th, c)][:, ts(tcl, QC)],
                            start=(c == 0),
                            stop=(c == CD - 1),
                        )
                    nc.vector.tensor_copy(
                        QT[:, et, ds(th * TH + tcl * QC, QC)], ps[:]
                    )

                def v_chain(tt):
                    th, ttl = tt // (TH // P), tt % (TH // P)
                    ps = p1pool.tile([P, E], F32, tag="pp")
                    for c in range(CD):
                        nc.tensor.matmul(
                            ps[:],
                            xts[(th, c)][:, ts(ttl, P)],
                            wv_sb[:, c, :],
                            start=(c == 0),
                            stop=(c == CD - 1),
                        )
                    nc.vector.tensor_copy(
                        V[:, tt, :, 0:HD],
                        ps[:].rearrange("p (h e) -> p h e", h=H),
                    )

                def y_group(et, tcq):
                    ps = p1pool.tile([P, QC], F32, tag="pp")
                    for fc in range(4):
                        nc.tensor.matmul(
                            ps[:],
                            wo_sb[:, fc, ts(et, P)],
                            outT[:, fc, ts(tcq, QC)],
                            start=(fc == 0),
                            stop=(fc == 3),
                        )
                    yt = yev.tile([P, QC], MDT, tag="ye")
                    nc.vector.tensor_copy(yt[:], ps[:])
                    nc.sync.dma_start(
                        out=yT[et * P : (et + 1) * P, ts(tcq, QC)], in_=yt[:]
                    )

                # ---------- phase-2 unit helpers ------------------------------
                # cell = kt*2 + parity; 16 segments of 2 cells
                SEGS = [(s, s + 2) for s in range(0, 32, 2)]

                def emit_score_seg(j, qc, c0, c1, Pu):
                    n = c1 - c0
                    sp = spool.tile([P, 2 * QC], F32, tag="S")
                    for u, cell in enumerate(range(c0, c1)):
                        kt, par = cell >> 1, cell & 1
                        off = par * HD
                        nc.tensor.matmul(
                            sp[:, u * QC : (u + 1) * QC],
                            KT[off : off + HD, j, kt * P : (kt + 1) * P],
                            QT[off : off + HD, j, ts(qc, QC)],
                            start=True,
                            stop=True,
                        )
                    nc.scalar.activation(
                        out=Pu[:, c0:c1, :].rearrange("p a b -> p (a b)"),
                        in_=sp[:, 0 : n * QC],
                        func=EXP,
                        scale=scale,
                    )

                def emit_av(j, par, opsum, Pu, kt):
                    h = 2 * j + par
                    nc.tensor.matmul(
                        opsum[0 : HD + 1, :],
                        V[:, kt, h, :],
                        Pu[:, 2 * kt + par, :],
                        start=(kt == 0),
                        stop=(kt == NKT - 1),
                    )

                def emit_finish(opsum, j, par, qc):
                    off = par * HD
                    den = nrmp.tile([1, QC], F32, tag="dn")
                    nc.vector.tensor_copy(den[:], opsum[HD : HD + 1, :])
                    recip = nrmp.tile([1, QC], F32, tag="rc")
                    nc.vector.reciprocal_approx_fast(recip[:], den[:])
                    bcs = nrmp.tile([HD, QC], F32, tag="bcs")
                    nc.gpsimd.partition_broadcast(bcs[:], recip[:], channels=HD)
                    nc.vector.tensor_mul(
                        outT[off : off + HD, j, ts(qc, QC)],
                        opsum[0:HD, :],
                        bcs[:],
                    )

                # ---------- static filler schedule ----------------------------
                # unit u = (qc, j) with qc outer: u = qc*4 + j.
                # Constraints encoded here: KT(et_j)/QT(et_j, qc) chains are
                # emitted at least one unit before the unit that consumes
                # them; V th0 before unit 1's early AV, V th1 within unit 1;
                # y(tcq) only after all 4 pairs of that tcq have finished
                # (AV of (tcq, j3) runs in the next unit).
                FILLER = {
                    0: [("kt", 0, 1, 0), ("kt", 0, 1, 1),
                        ("kt", 1, 0, 0), ("kt", 1, 0, 1), ("kt", 1, 1, 0),
                        ("kt", 1, 1, 1), ("qt", 1, 0, 0),
                        ("v", 0), ("v", 1), ("v", 2), ("v", 3),
                        ("v", 4), ("v", 5), ("v", 6), ("v", 7)],
                    1: [("v", 8), ("v", 9), ("v", 10), ("v", 11),
                        ("v", 12), ("v", 13), ("v", 14), ("v", 15),
                        ("kt", 2, 0, 0), ("kt", 2, 0, 1), ("kt", 2, 1, 0),
                        ("kt", 2, 1, 1), ("qt", 2, 0, 0)],
                    2: [("kt", 3, 0, 0), ("kt", 3, 0, 1), ("kt", 3, 1, 0),
                        ("kt", 3, 1, 1), ("qt", 3, 0, 0), ("qt", 0, 0, 1)],
                    3: [("qt", 1, 0, 1), ("qt", 2, 0, 1), ("qt", 3, 0, 1)],
                    4: [("qt", 0, 1, 0), ("qt", 1, 1, 0)],
                    5: [("qt", 2, 1, 0), ("qt", 3, 1, 0),
                        ("y", 0, 0), ("y", 1, 0)],
                    6: [("qt", 0, 1, 1), ("qt", 1, 1, 1),
                        ("y", 2, 0), ("y", 3, 0), ("y", 4, 0)],
                    7: [("qt", 2, 1, 1), ("qt", 3, 1, 1),
                        ("y", 5, 0), ("y", 6, 0), ("y", 7, 0)],
                    8: [],
                    9: [("y", 0, 1), ("y", 1, 1), ("y", 2, 1)],
                    10: [("y", 3, 1), ("y", 4, 1), ("y", 5, 1)],
                    11: [("y", 6, 1), ("y", 7, 1)],
                    12: [],
                    13: [("y", 0, 2), ("y", 1, 2), ("y", 2, 2)],
                    14: [("y", 3, 2), ("y", 4, 2), ("y", 5, 2)],
                    15: [("y", 6, 2), ("y", 7, 2)],
                }

                def emit_filler(item):
                    kind = item[0]
                    if kind == "kt":
                        kt_chain(*item[1:])
                    elif kind == "qt":
                        qt_chain(*item[1:])
                    elif kind == "v":
                        v_chain(item[1])
                    else:
                        y_group(item[1], item[2])

                # ---------- lead-in: only what unit 0's first cells need ------
                # (cells 0..15 are kt 0-7 = th0 keys; th1 KT chains follow in
                # unit 0's filler before cell 16 is reached)
                kt_chain(0, 0, 0)
                kt_chain(0, 0, 1)
                qt_chain(0, 0, 0)

                # ---------- the unit loop -------------------------------------
                units = [(qc, j) for qc in range(NQC) for j in range(NP)]
                # Segments are emitted in PAIRS (both psum slots fill
                # back-to-back) so the e/o score alternation is unbroken for
                # 4 cells and only one full-drain wait is paid per 4 cells.
                NDS = len(SEGS) // 2
                # AV-MM spread across the 8 double-steps (32 per unit)
                SPREAD = [4] * NDS
                # unit 1 consumes V-th1 chains first, AV back-loaded
                SPREAD_U1 = [0, 0, 6, 6, 6, 6, 4, 4]

                prev = None  # (opsum_e, opsum_o, Pu, j, qc)
                for ui, (qc, j) in enumerate(units):
                    Pu = pbuf.tile([P, 2 * NKT, QC], MDT, tag="P")
                    filler = list(FILLER.get(ui, []))
                    # split filler across segments roughly evenly
                    nf = len(filler)
                    spread = SPREAD_U1 if ui == 1 else SPREAD
                    av_i = 0
                    for si in range(NDS):
                        emit_score_seg(j, qc, 4 * si, 4 * si + 2, Pu)
                        emit_score_seg(j, qc, 4 * si + 2, 4 * si + 4, Pu)
                        if prev is not None:
                            pe, po, pPu, pj, pqc = prev
                            for _ in range(spread[si]):
                                if av_i >= 2 * NKT:
                                    break
                                kt, par = av_i >> 1, av_i & 1
                                emit_av(pj, par, pe if par == 0 else po,
                                        pPu, kt)
                                av_i += 1
                        # one filler chain after each segment, round-robin
                        want = (nf * (si + 1)) // NDS
                        while len(filler) > nf - want and filler:
                            emit_filler(filler.pop(0))
                    if prev is not None:
                        pe, po, pPu, pj, pqc = prev
                        while av_i < 2 * NKT:
                            kt, par = av_i >> 1, av_i & 1
                            emit_av(pj, par, pe if par == 0 else po, pPu, kt)
                            av_i += 1
                        emit_finish(pe, pj, 0, pqc)
                        emit_finish(po, pj, 1, pqc)
                    opsum_e = ovpool.tile([P, QC], F32, tag="ov")
                    opsum_o = ovpool.tile([P, QC], F32, tag="ov")
                    prev = (opsum_e, opsum_o, Pu, j, qc)

                # ---------- tail ----------------------------------------------
                pe, po, pPu, pj, pqc = prev
                for kt in range(NKT):
                    emit_av(pj, 0, pe, pPu, kt)
                    emit_av(pj, 1, po, pPu, kt)
                emit_finish(pe, pj, 0, pqc)
                emit_finish(po, pj, 1, pqc)
                for et in range(DM // P):
                    y_group(et, 3)

    nc.compile()
    return nc


_CACHE: dict = {}


def _get_program(scale: float):
    key = round(float(scale), 12)
    if key not in _CACHE:
        _CACHE[key] = build(key)
    return _CACHE[key]


def _make_in_maps(x, w_qkv, w_out):
    cdt = np.float16
    xTs = [np.ascontiguousarray(x[b].T).astype(cdt) for b in range(4)]
    wslices = []
    for hg in range(2):
        sl = slice(hg * E, (hg + 1) * E)
        wslices.append(
            {
                "wq": np.ascontiguousarray(w_qkv[0 * DM :][sl, :].T).astype(cdt),
                "wk": np.ascontiguousarray(w_qkv[1 * DM :][sl, :].T).astype(cdt),
                "wv": np.ascontiguousarray(w_qkv[2 * DM :][sl, :].T).astype(cdt),
                "wo": np.ascontiguousarray(w_out[:, sl].T).astype(cdt),
            }
        )
    in_maps = []
    for c in range(8):
        b, hg = c // 2, c % 2
        m = {"xT": xTs[b]}
        m.update(wslices[hg])
        in_maps.append(m)
    return in_maps


def _execute(x, w_qkv, w_out, rescale, **spmd_kwargs):
    scale = float(np.asarray(rescale)) / math.sqrt(HD)
    nc = _get_program(scale)
    in_maps = _make_in_maps(x, w_qkv, w_out)
    return run_bass_kernel_spmd(nc, in_maps, list(range(8)), **spmd_kwargs)


def kernel(x, w_qkv, w_out, b_out, rescale):
    x = np.asarray(x, dtype=np.float32)
    w_qkv = np.asarray(w_qkv, dtype=np.float32)
    w_out = np.asarray(w_out, dtype=np.float32)
    b_out = np.asarray(b_out, dtype=np.float32)
    res = _execute(x, w_qkv, w_out, rescale).results
    y = np.empty((4, T, DM), dtype=np.float32)
    for b in range(4):
        acc = res[2 * b]["yT"].astype(np.float32) + res[2 * b + 1]["yT"].astype(
            np.float32
        )
        y[b] = acc.T + b_out
    return y


# revision 22
# speedup vs baseline: 1.0974x; 1.0003x over previous
"""DriftAwareMultiHeadAttention on 8 Trainium2 NeuronCores.

Sharding (per spec hint): core c -> (batch b = c//2, head-group hg = c%2).
Each core runs the QKV projection column-parallel over its 8 heads, full
attention for those heads, and a row-parallel partial output projection.
Host gather: y[b] = (yT[2b] + yT[2b+1]).T + b_out.

v2: paired-head phase 2.  The per-core 8 heads are processed as 4 pairs
(2j, 2j+1); the pair's K^T/Q^T live in the same e-tile at partition
offsets 0/64, so their K=64 score matmuls carry tile_position (0,0) and
(64,0) and run CONCURRENTLY on disjoint row-groups of the PE array
(2 matmuls per ~N cycles instead of 1).  This removes the half-array
waste of the hd=64 contraction: scores cost ~56us instead of ~110us.

Device layout is feature-on-partition / token-on-free throughout:
  - Q^T, K^T: [512, 2048] fp16 (pair j -> e-tile j, head parity ->
    partition offset 0/64).  V: [tokens, 8 heads x (64+1)] fp16 -- the
    extra "ones" column makes the AV matmul emit the softmax denominator
    in psum row 64 for free.
  - Unit (qc, j): 32 cells (cell = kt*2 + parity) of S^T score matmuls,
    emitted in cell order so adjacent matmuls alternate row-groups.
    Segments of 3 cells -> one [128, 1536] fp32 psum tile (3 banks,
    double-buffered = 6) -> ONE flat-2D-AP exp per segment (ScalarE,
    score scale folded in) -> Pu cells [128, 32, 512] fp16.
    Flat 2D APs on the exp are load-bearing: 3D/strided APs cost ~2x
    instruction overhead on ACT and also degrade PE issue spacing.
  - AV accumulates [65, 512] fp32 over 16 k-tiles (1 bank per head, 2
    banks per pair; psum total = 6+2 = 8 banks exactly).  Row 64 is the
    denominator: DVE reciprocal_approx_fast + GpSimd partition_broadcast
    + DVE multiply into outT.
  - output projection y^T = wo^T @ outT in fp16 with fp32 psum, emitted
    as 4-matmul groups borrowing a score-psum rotation slot.

Phase-1 projections are folded into the phase-2 unit stream as filler:
only K^T(et0) + Q^T(et0,qc0) precede unit 0 (~12us lead-in), everything
else (V, remaining K^T/Q^T chains, y-groups) fills the PE while ScalarE
drains exp segments.  exp starts ~12us into the kernel instead of ~75us.

PSUM budget: scores 2x[128,3x512] (6 banks) + AV 2x[128,512] (2 banks).
fp16 everywhere 16-bit (same matmul speed as bf16, 8x lower rounding
error on these O(1) tensors); fp32 psum.
"""

import math

import numpy as np

import concourse.bass as bass
import concourse.mybir as mybir
import concourse.tile as tile
from concourse import bacc
from concourse.bass import ds, ts
from concourse.bass_utils import run_bass_kernel_spmd

P = 128
T = 2048        # tokens per batch
DM = 1024       # model dim
E = 512         # per-core projection width (8 heads * 64)
H = 8           # heads per core
HD = 64
CD = DM // P    # contraction chunks over model dim
NKT = T // P    # k tiles per head
QC = 512        # q chunk
NQC = T // QC
TH = T // 2
NP = H // 2     # head pairs
F32 = mybir.dt.float32
FP16 = mybir.dt.float16
EXP = mybir.ActivationFunctionType.Exp


def build(scale: float):
    MDT = FP16
    nc = bacc.Bacc(None, target_bir_lowering=False, debug=False)
    xT = nc.declare_dram_parameter("xT", [DM, T], MDT, isOutput=False)
    wq = nc.declare_dram_parameter("wq", [DM, E], MDT, isOutput=False)
    wk = nc.declare_dram_parameter("wk", [DM, E], MDT, isOutput=False)
    wv = nc.declare_dram_parameter("wv", [DM, E], MDT, isOutput=False)
    wo = nc.declare_dram_parameter("wo", [E, DM], MDT, isOutput=False)
    yT = nc.declare_dram_parameter("yT", [DM, T], MDT, isOutput=True)

    with tile.TileContext(nc) as tc:
        with (
            tc.tile_pool(name="qk", bufs=1) as qkp,
            tc.tile_pool(name="vp", bufs=1) as vp,
            tc.tile_pool(name="misc", bufs=1) as miscp,
            tc.tile_pool(name="wts", bufs=1) as wp,
            tc.tile_pool(name="xt", bufs=1) as xp,
        ):
            QT = qkp.tile([P, 4, T], MDT, tag="QT")
            KT = qkp.tile([P, 4, T], MDT, tag="KT")
            V = vp.tile([P, NKT, H, HD + 1], MDT, tag="V")
            nc.vector.memset(V[:, :, :, HD : HD + 1], 1.0)
            # preload the exp table set so the first real exp doesn't stall
            warm = miscp.tile([1, 8], F32, tag="warm")
            nc.vector.memset(warm[:], 0.0)
            nc.scalar.activation(out=warm[:], in_=warm[:], func=EXP, scale=1.0)

            wq_sb = wp.tile([P, CD, E], MDT, tag="wq")
            wk_sb = wp.tile([P, CD, E], MDT, tag="wk")
            wv_sb = wp.tile([P, CD, E], MDT, tag="wv")
            # DMA order is the critical path to the first exp: the first
            # unit's cells 0..15 touch only th0 tokens, so KT(et0,th0) +
            # QT(et0,th0,tcl0) gate it -> wk, x(th0), wq first; x(th1), wv
            # after.
            xts = {}

            def _dma_x(th):
                for c in range(CD):
                    xt = xp.tile([P, TH], MDT, tag=f"x{th}_{c}")
                    nc.sync.dma_start(
                        out=xt[:],
                        in_=xT[c * P : (c + 1) * P, th * TH : (th + 1) * TH],
                    )
                    xts[(th, c)] = xt

            # Interleave wk/x(th0) chunk DMAs so the first KT chain's matmul
            # for chunk c can start as soon as chunk c lands.
            wk_r = wk.rearrange("(c p) e -> p c e", p=P)
            for c in range(CD):
                nc.sync.dma_start(out=wk_sb[:, c, :], in_=wk_r[:, c, :])
                xt = xp.tile([P, TH], MDT, tag=f"x0_{c}")
                nc.sync.dma_start(
                    out=xt[:], in_=xT[c * P : (c + 1) * P, 0:TH]
                )
                xts[(0, c)] = xt
            wq_r = wq.rearrange("(c p) e -> p c e", p=P)
            for c in range(CD):
                nc.sync.dma_start(out=wq_sb[:, c, :], in_=wq_r[:, c, :])
            _dma_x(1)
            nc.sync.dma_start(out=wv_sb[:], in_=wv.rearrange("(c p) e -> p c e", p=P))

            with (
                tc.tile_pool(name="outp", bufs=1) as outp,
                tc.tile_pool(name="pbuf", bufs=2) as pbuf,
                tc.tile_pool(name="nrm", bufs=2) as nrmp,
                tc.tile_pool(name="wop", bufs=1) as wop,
                tc.tile_pool(name="yev", bufs=3) as yev,
                tc.tile_pool(name="sps", bufs=2, space="PSUM") as spool,
                tc.tile_pool(name="ovp", bufs=2, space="PSUM") as ovpool,
                tc.tile_pool(name="p1", bufs=2, space="PSUM") as p1pool,
            ):
                outT = outp.tile([P, 4, T], MDT, tag="outT")
                wo_sb = wop.tile([P, 4, DM], MDT, tag="wo")
                nc.sync.dma_start(
                    out=wo_sb[:], in_=wo.rearrange("(c p) e -> p c e", p=P)
                )

                # ---------- filler chain emitters (each: one 8/4-MM chain) ----
                # Filler runs in its own double-buffered 2-bank psum pool so
                # it never perturbs the score-segment double-buffering.
                # Total psum: scores 2x2 + filler 2 + AV 2 = 8 banks.
                def kt_chain(et, th, tcl):
                    ps = p1pool.tile([P, QC], F32, tag="pp")
                    for c in range(CD):
                        nc.tensor.matmul(
                            ps[:],
                            wk_sb[:, c, ts(et, P)],
                            xts[(th, c)][:, ts(tcl, QC)],
                            start=(c == 0),
                            stop=(c == CD - 1),
                        )
                    nc.vector.tensor_copy(
                        KT[:, et, ds(th * TH + tcl * QC, QC)], ps[:]
                    )

                def qt_chain(et, th, tcl):
                    ps = p1pool.tile([P, QC], F32, tag="pp")
                    for c in range(CD):
                        nc.tensor.matmul(
                            ps[:],
                            wq_sb[:, c, ts(et, P)],
                            xts[(th, c)][:, ts(tcl, QC)],
                            start=(c == 0),
                            stop=(c == CD - 1),
                        )
                    nc.vector.tensor_copy(
                        QT[:, et, ds(th * TH + tcl * QC, QC)], ps[:]
                    )

                def v_chain(tt):
                    th, ttl = tt // (TH // P), tt % (TH // P)
                    ps = p1pool.tile([P, E], F32, tag="pp")
                    for c in range(CD):
                        nc.tensor.matmul(
                            ps[:],
                            xts[(th, c)][:, ts(ttl, P)],
                            wv_sb[:, c, :],
                            start=(c == 0),
                            stop=(c == CD - 1),
                        )
                    nc.vector.tensor_copy(
                        V[:, tt, :, 0:HD],
                        ps[:].rearrange("p (h e) -> p h e", h=H),
                    )

                def y_group(et, tcq):
                    ps = p1pool.tile([P, QC], F32, tag="pp")
                    for fc in range(4):
                        nc.tensor.matmul(
                            ps[:],
                            wo_sb[:, fc, ts(et, P)],
                            outT[:, fc, ts(tcq, QC)],
                            start=(fc == 0),
                            stop=(fc == 3),
                        )
                    yt = yev.tile([P, QC], MDT, tag="ye")
                    nc.vector.tensor_copy(yt[:], ps[:])
                    nc.sync.dma_start(
                        out=yT[et * P : (et + 1) * P, ts(tcq, QC)], in_=yt[:]
                    )

                # ---------- phase-2 unit helpers ------------------------------
                # cell = kt*2 + parity; 16 segments of 2 cells
                SEGS = [(s, s + 2) for s in range(0, 32, 2)]

                def emit_score_seg(j, qc, c0, c1, Pu):
                    n = c1 - c0
                    sp = spool.tile([P, 2 * QC], F32, tag="S")
                    for u, cell in enumerate(range(c0, c1)):
                        kt, par = cell >> 1, cell & 1
                        off = par * HD
                        nc.tensor.matmul(
                            sp[:, u * QC : (u + 1) * QC],
                            KT[off : off + HD, j, kt * P : (kt + 1) * P],
                            QT[off : off + HD, j, ts(qc, QC)],
                            start=True,
                            stop=True,
                        )
                    nc.scalar.activation(
                        out=Pu[:, c0:c1, :].rearrange("p a b -> p (a b)"),
                        in_=sp[:, 0 : n * QC],
                        func=EXP,
                        scale=scale,
                    )

                def emit_av(j, par, opsum, Pu, kt):
                    h = 2 * j + par
                    nc.tensor.matmul(
                        opsum[0 : HD + 1, :],
                        V[:, kt, h, :],
                        Pu[:, 2 * kt + par, :],
                        start=(kt == 0),
                        stop=(kt == NKT - 1),
                    )

                def emit_finish(opsum, j, par, qc):
                    off = par * HD
                    den = nrmp.tile([1, QC], F32, tag="dn")
                    nc.vector.tensor_copy(den[:], opsum[HD : HD + 1, :])
                    recip = nrmp.tile([1, QC], F32, tag="rc")
                    nc.vector.reciprocal_approx_fast(recip[:], den[:])
                    bcs = nrmp.tile([HD, QC], F32, tag="bcs")
                    nc.gpsimd.partition_broadcast(bcs[:], recip[:], channels=HD)
                    nc.vector.tensor_mul(
                        outT[off : off + HD, j, ts(qc, QC)],
                        opsum[0:HD, :],
                        bcs[:],
                    )

                # ---------- static filler schedule ----------------------------
                # unit u = (qc, j) with qc outer: u = qc*4 + j.
                # Constraints encoded here: KT(et_j)/QT(et_j, qc) chains are
                # emitted at least one unit before the unit that consumes
                # them; V th0 before unit 1's early AV, V th1 within unit 1;
                # y(tcq) only after all 4 pairs of that tcq have finished
                # (AV of (tcq, j3) runs in the next unit).
                FILLER = {
                    0: [("kt", 0, 1, 0), ("kt", 0, 1, 1),
                        ("kt", 1, 0, 0), ("kt", 1, 0, 1), ("kt", 1, 1, 0),
                        ("kt", 1, 1, 1), ("qt", 1, 0, 0),
                        ("v", 0), ("v", 1), ("v", 2), ("v", 3),
                        ("v", 4), ("v", 5), ("v", 6), ("v", 7)],
                    1: [("v", 8), ("v", 9), ("v", 10), ("v", 11),
                        ("v", 12), ("v", 13), ("v", 14), ("v", 15),
                        ("kt", 2, 0, 0), ("kt", 2, 0, 1), ("kt", 2, 1, 0),
                        ("kt", 2, 1, 1), ("qt", 2, 0, 0)],
                    2: [("kt", 3, 0, 0), ("kt", 3, 0, 1), ("kt", 3, 1, 0),
                        ("kt", 3, 1, 1), ("qt", 3, 0, 0), ("qt", 0, 0, 1)],
                    3: [("qt", 1, 0, 1), ("qt", 2, 0, 1), ("qt", 3, 0, 1)],
                    4: [("qt", 0, 1, 0), ("qt", 1, 1, 0)],
                    5: [("qt", 2, 1, 0), ("qt", 3, 1, 0),
                        ("y", 0, 0), ("y", 1, 0)],
                    6: [("qt", 0, 1, 1), ("qt", 1, 1, 1),
                        ("y", 2, 0), ("y", 3, 0), ("y", 4, 0)],
                    7: [("qt", 2, 1, 1), ("qt", 3, 1, 1),
                        ("y", 5, 0), ("y", 6, 0), ("y", 7, 0)],
                    8: [],
                    9: [("y", 0, 1), ("y", 1, 1), ("y", 2, 1)],
                    10: [("y", 3, 1), ("y", 4, 1), ("y", 5, 1)],
                    11: [("y", 6, 1), ("y", 7, 1)],
                    12: [],
                    13: [("y", 0, 2), ("y", 1, 2), ("y", 2, 2)],
                    14: [("y", 3, 2), ("y", 4, 2), ("y", 5, 2),
                         ("y", 6, 2), ("y", 7, 2)],
                    15: [],
                }

                def emit_filler(item):
                    kind = item[0]
                    if kind == "kt":
                        kt_chain(*item[1:])
                    elif kind == "qt":
                        qt_chain(*item[1:])
                    elif kind == "v":
                        v_chain(item[1])
                    else:
                        y_group(item[1], item[2])

                # ---------- lead-in: only what unit 0's first cells need ------
                # (cells 0..15 are kt 0-7 = th0 keys; th1 KT chains follow in
                # unit 0's filler before cell 16 is reached)
                kt_chain(0, 0, 0)
                kt_chain(0, 0, 1)
                qt_chain(0, 0, 0)

                # ---------- the unit loop -------------------------------------
                units = [(qc, j) for qc in range(NQC) for j in range(NP)]
                # Segments are emitted in PAIRS (both psum slots fill
                # back-to-back) so the e/o score alternation is unbroken for
                # 4 cells and only one full-drain wait is paid per 4 cells.
                NDS = len(SEGS) // 2
                # AV-MM spread across the 8 double-steps (32 per unit)
                SPREAD = [4] * NDS
                # unit 1 consumes V-th1 chains first, AV back-loaded
                SPREAD_U1 = [0, 0, 6, 6, 6, 6, 4, 4]

                prev = None  # (opsum_e, opsum_o, Pu, j, qc)
                last_ui = len(units) - 1
                self_av = 0
                for ui, (qc, j) in enumerate(units):
                    Pu = pbuf.tile([P, 2 * NKT, QC], MDT, tag="P")
                    if ui == last_ui:
                        # the last unit's AV trails its own exps inside the
                        # segment loop; its accumulators live in the (now
                        # idle) filler psum pool since ovpool's banks are
                        # still held by the previous unit's AV.
                        own_e = p1pool.tile([P, QC], F32, tag="pp")
                        own_o = p1pool.tile([P, QC], F32, tag="pp")
                    filler = list(FILLER.get(ui, []))
                    # split filler across segments roughly evenly
                    nf = len(filler)
                    spread = SPREAD_U1 if ui == 1 else SPREAD
                    av_i = 0
                    for si in range(NDS):
                        emit_score_seg(j, qc, 4 * si, 4 * si + 2, Pu)
                        emit_score_seg(j, qc, 4 * si + 2, 4 * si + 4, Pu)
                        if prev is not None:
                            pe, po, pPu, pj, pqc = prev
                            for _ in range(spread[si]):
                                if av_i >= 2 * NKT:
                                    break
                                kt, par = av_i >> 1, av_i & 1
                                emit_av(pj, par, pe if par == 0 else po,
                                        pPu, kt)
                                av_i += 1
                        if ui == last_ui:
                            while self_av < min(4 * (si - 1), 2 * NKT):
                                kt, par = self_av >> 1, self_av & 1
                                emit_av(j, par, own_e if par == 0 else own_o,
                                        Pu, kt)
                                self_av += 1
                        # one filler chain after each segment, round-robin
                        want = (nf * (si + 1)) // NDS
                        while len(filler) > nf - want and filler:
                            emit_filler(filler.pop(0))
                    if prev is not None:
                        pe, po, pPu, pj, pqc = prev
                        while av_i < 2 * NKT:
                            kt, par = av_i >> 1, av_i & 1
                            emit_av(pj, par, pe if par == 0 else po, pPu, kt)
                            av_i += 1
                        emit_finish(pe, pj, 0, pqc)
                        emit_finish(po, pj, 1, pqc)
                    if ui != last_ui:
                        opsum_e = ovpool.tile([P, QC], F32, tag="ov")
                        opsum_o = ovpool.tile([P, QC], F32, tag="ov")
                        prev = (opsum_e, opsum_o, Pu, j, qc)
                    else:
                        prev = (own_e, own_o, Pu, j, qc)

                # ---------- tail ----------------------------------------------
                pe, po, pPu, pj, pqc = prev
                while self_av < 2 * NKT:
                    kt, par = self_av >> 1, self_av & 1
                    emit_av(pj, par, pe if par == 0 else po, pPu, kt)
                    self_av += 1
                emit_finish(pe, pj, 0, pqc)
                emit_finish(po, pj, 1, pqc)
                for et in range(DM // P):
                    y_group(et, 3)

    nc.compile()
    return nc


_CACHE: dict = {}


def _get_program(scale: float):
    key = round(float(scale), 12)
    if key not in _CACHE:
        _CACHE[key] = build(key)
    return _CACHE[key]


def _make_in_maps(x, w_qkv, w_out):
    cdt = np.float16
    xTs = [np.ascontiguousarray(x[b].T).astype(cdt) for b in range(4)]
    wslices = []
    for hg in range(2):
        sl = slice(hg * E, (hg + 1) * E)
        wslices.append(
            {
                "wq": np.ascontiguousarray(w_qkv[0 * DM :][sl, :].T).astype(cdt),
                "wk": np.ascontiguousarray(w_qkv[1 * DM :][sl, :].T).astype(cdt),
                "wv": np.ascontiguousarray(w_qkv[2 * DM :][sl, :].T).astype(cdt),
                "wo": np.ascontiguousarray(w_out[:, sl].T).astype(cdt),
            }
        )
    in_maps = []
    for c in range(8):
        b, hg = c // 2, c % 2
        m = {"xT": xTs[b]}
        m.update(wslices[hg])
        in_maps.append(m)
    return in_maps


def _execute(x, w_qkv, w_out, rescale, **spmd_kwargs):
    scale = float(np.asarray(rescale)) / math.sqrt(HD)
    nc = _get_program(scale)
    in_maps = _make_in_maps(x, w_qkv, w_out)
    return run_bass_kernel_spmd(nc, in_maps, list(range(8)), **spmd_kwargs)


def kernel(x, w_qkv, w_out, b_out, rescale):
    x = np.asarray(x, dtype=np.float32)
    w_qkv = np.asarray(w_qkv, dtype=np.float32)
    w_out = np.asarray(w_out, dtype=np.float32)
    b_out = np.asarray(b_out, dtype=np.float32)
    res = _execute(x, w_qkv, w_out, rescale).results
    y = np.empty((4, T, DM), dtype=np.float32)
    for b in range(4):
        acc = res[2 * b]["yT"].astype(np.float32) + res[2 * b + 1]["yT"].astype(
            np.float32
        )
        y[b] = acc.T + b_out
    return y


# revision 24
# speedup vs baseline: 1.1033x; 1.0054x over previous
"""DriftAwareMultiHeadAttention on 8 Trainium2 NeuronCores.

Sharding (per spec hint): core c -> (batch b = c//2, head-group hg = c%2).
Each core runs the QKV projection column-parallel over its 8 heads, full
attention for those heads, and a row-parallel partial output projection.
Host gather: y[b] = (yT[2b] + yT[2b+1]).T + b_out.

v2: paired-head phase 2.  The per-core 8 heads are processed as 4 pairs
(2j, 2j+1); the pair's K^T/Q^T live in the same e-tile at partition
offsets 0/64, so their K=64 score matmuls carry tile_position (0,0) and
(64,0) and run CONCURRENTLY on disjoint row-groups of the PE array
(2 matmuls per ~N cycles instead of 1).  This removes the half-array
waste of the hd=64 contraction: scores cost ~56us instead of ~110us.

Device layout is feature-on-partition / token-on-free throughout:
  - Q^T, K^T: [512, 2048] fp16 (pair j -> e-tile j, head parity ->
    partition offset 0/64).  V: [tokens, 8 heads x (64+1)] fp16 -- the
    extra "ones" column makes the AV matmul emit the softmax denominator
    in psum row 64 for free.
  - Unit (qc, j): 32 cells (cell = kt*2 + parity) of S^T score matmuls,
    emitted in cell order so adjacent matmuls alternate row-groups.
    Segments of 3 cells -> one [128, 1536] fp32 psum tile (3 banks,
    double-buffered = 6) -> ONE flat-2D-AP exp per segment (ScalarE,
    score scale folded in) -> Pu cells [128, 32, 512] fp16.
    Flat 2D APs on the exp are load-bearing: 3D/strided APs cost ~2x
    instruction overhead on ACT and also degrade PE issue spacing.
  - AV accumulates [65, 512] fp32 over 16 k-tiles (1 bank per head, 2
    banks per pair; psum total = 6+2 = 8 banks exactly).  Row 64 is the
    denominator: DVE reciprocal_approx_fast + GpSimd partition_broadcast
    + DVE multiply into outT.
  - output projection y^T = wo^T @ outT in fp16 with fp32 psum, emitted
    as 4-matmul groups borrowing a score-psum rotation slot.

Phase-1 projections are folded into the phase-2 unit stream as filler:
only K^T(et0) + Q^T(et0,qc0) precede unit 0 (~12us lead-in), everything
else (V, remaining K^T/Q^T chains, y-groups) fills the PE while ScalarE
drains exp segments.  exp starts ~12us into the kernel instead of ~75us.

PSUM budget: scores 2x[128,3x512] (6 banks) + AV 2x[128,512] (2 banks).
fp16 everywhere 16-bit (same matmul speed as bf16, 8x lower rounding
error on these O(1) tensors); fp32 psum.
"""

import math

import numpy as np

import concourse.bass as bass
import concourse.mybir as mybir
import concourse.tile as tile
from concourse import bacc
from concourse.bass import ds, ts
from concourse.bass_utils import run_bass_kernel_spmd

P = 128
T = 2048        # tokens per batch
DM = 1024       # model dim
E = 512         # per-core projection width (8 heads * 64)
H = 8           # heads per core
HD = 64
CD = DM // P    # contraction chunks over model dim
NKT = T // P    # k tiles per head
QC = 512        # q chunk
NQC = T // QC
TH = T // 2
NP = H // 2     # head pairs
F32 = mybir.dt.float32
FP16 = mybir.dt.float16
EXP = mybir.ActivationFunctionType.Exp


def build(scale: float):
    MDT = FP16
    nc = bacc.Bacc(None, target_bir_lowering=False, debug=False)
    xT = nc.declare_dram_parameter("xT", [DM, T], MDT, isOutput=False)
    wq = nc.declare_dram_parameter("wq", [DM, E], MDT, isOutput=False)
    wk = nc.declare_dram_parameter("wk", [DM, E], MDT, isOutput=False)
    wv = nc.declare_dram_parameter("wv", [DM, E], MDT, isOutput=False)
    wo = nc.declare_dram_parameter("wo", [E, DM], MDT, isOutput=False)
    yT = nc.declare_dram_parameter("yT", [DM, T], MDT, isOutput=True)

    with tile.TileContext(nc) as tc:
        with (
            tc.tile_pool(name="qk", bufs=1) as qkp,
            tc.tile_pool(name="vp", bufs=1) as vp,
            tc.tile_pool(name="misc", bufs=1) as miscp,
            tc.tile_pool(name="wts", bufs=1) as wp,
            tc.tile_pool(name="xt", bufs=1) as xp,
        ):
            QT = qkp.tile([P, 4, T], MDT, tag="QT")
            KT = qkp.tile([P, 4, T], MDT, tag="KT")
            V = vp.tile([P, NKT, H, HD + 1], MDT, tag="V")
            nc.vector.memset(V[:, :, :, HD : HD + 1], 1.0)
            # preload the exp table set so the first real exp doesn't stall
            warm = miscp.tile([1, 8], F32, tag="warm")
            nc.vector.memset(warm[:], 0.0)
            nc.scalar.activation(out=warm[:], in_=warm[:], func=EXP, scale=1.0)

            wq_sb = wp.tile([P, CD, E], MDT, tag="wq")
            wk_sb = wp.tile([P, CD, E], MDT, tag="wk")
            wv_sb = wp.tile([P, CD, E], MDT, tag="wv")
            # DMA order is the critical path to the first exp: the first
            # unit's cells 0..15 touch only th0 tokens, so KT(et0,th0) +
            # QT(et0,th0,tcl0) gate it -> wk, x(th0), wq first; x(th1), wv
            # after.
            xts = {}

            def _dma_x(th):
                for c in range(CD):
                    xt = xp.tile([P, TH], MDT, tag=f"x{th}_{c}")
                    nc.sync.dma_start(
                        out=xt[:],
                        in_=xT[c * P : (c + 1) * P, th * TH : (th + 1) * TH],
                    )
                    xts[(th, c)] = xt

            # Interleave wk/x(th0) chunk DMAs so the first KT chain's matmul
            # for chunk c can start as soon as chunk c lands, and spread the
            # input streams across several engines' DGE queues so the ramp
            # isn't serialized behind one queue.
            wk_r = wk.rearrange("(c p) e -> p c e", p=P)
            wq_r = wq.rearrange("(c p) e -> p c e", p=P)
            for c in range(CD):
                nc.scalar.dma_start(out=wk_sb[:, c, :], in_=wk_r[:, c, :])
                xt = xp.tile([P, TH], MDT, tag=f"x0_{c}")
                nc.sync.dma_start(
                    out=xt[:], in_=xT[c * P : (c + 1) * P, 0:TH]
                )
                xts[(0, c)] = xt
                nc.gpsimd.dma_start(out=wq_sb[:, c, :], in_=wq_r[:, c, :])
            _dma_x(1)
            nc.sync.dma_start(out=wv_sb[:], in_=wv.rearrange("(c p) e -> p c e", p=P))

            with (
                tc.tile_pool(name="outp", bufs=1) as outp,
                tc.tile_pool(name="pbuf", bufs=2) as pbuf,
                tc.tile_pool(name="nrm", bufs=2) as nrmp,
                tc.tile_pool(name="wop", bufs=1) as wop,
                tc.tile_pool(name="yev", bufs=3) as yev,
                tc.tile_pool(name="sps", bufs=2, space="PSUM") as spool,
                tc.tile_pool(name="ovp", bufs=2, space="PSUM") as ovpool,
                tc.tile_pool(name="p1", bufs=2, space="PSUM") as p1pool,
            ):
                outT = outp.tile([P, 4, T], MDT, tag="outT")
                wo_sb = wop.tile([P, 4, DM], MDT, tag="wo")
                nc.sync.dma_start(
                    out=wo_sb[:], in_=wo.rearrange("(c p) e -> p c e", p=P)
                )

                # ---------- filler chain emitters (each: one 8/4-MM chain) ----
                # Filler runs in its own double-buffered 2-bank psum pool so
                # it never perturbs the score-segment double-buffering.
                # Total psum: scores 2x2 + filler 2 + AV 2 = 8 banks.
                def kt_chain(et, th, tcl):
                    ps = p1pool.tile([P, QC], F32, tag="pp")
                    for c in range(CD):
                        nc.tensor.matmul(
                            ps[:],
                            wk_sb[:, c, ts(et, P)],
                            xts[(th, c)][:, ts(tcl, QC)],
                            start=(c == 0),
                            stop=(c == CD - 1),
                        )
                    nc.vector.tensor_copy(
                        KT[:, et, ds(th * TH + tcl * QC, QC)], ps[:]
                    )

                def qt_chain(et, th, tcl):
                    ps = p1pool.tile([P, QC], F32, tag="pp")
                    for c in range(CD):
                        nc.tensor.matmul(
                            ps[:],
                            wq_sb[:, c, ts(et, P)],
                            xts[(th, c)][:, ts(tcl, QC)],
                            start=(c == 0),
                            stop=(c == CD - 1),
                        )
                    nc.vector.tensor_copy(
                        QT[:, et, ds(th * TH + tcl * QC, QC)], ps[:]
                    )

                def v_chain(tt):
                    th, ttl = tt // (TH // P), tt % (TH // P)
                    ps = p1pool.tile([P, E], F32, tag="pp")
                    for c in range(CD):
                        nc.tensor.matmul(
                            ps[:],
                            xts[(th, c)][:, ts(ttl, P)],
                            wv_sb[:, c, :],
                            start=(c == 0),
                            stop=(c == CD - 1),
                        )
                    nc.vector.tensor_copy(
                        V[:, tt, :, 0:HD],
                        ps[:].rearrange("p (h e) -> p h e", h=H),
                    )

                def y_group(et, tcq):
                    ps = p1pool.tile([P, QC], F32, tag="pp")
                    for fc in range(4):
                        nc.tensor.matmul(
                            ps[:],
                            wo_sb[:, fc, ts(et, P)],
                            outT[:, fc, ts(tcq, QC)],
                            start=(fc == 0),
                            stop=(fc == 3),
                        )
                    yt = yev.tile([P, QC], MDT, tag="ye")
                    nc.vector.tensor_copy(yt[:], ps[:])
                    nc.sync.dma_start(
                        out=yT[et * P : (et + 1) * P, ts(tcq, QC)], in_=yt[:]
                    )

                # ---------- phase-2 unit helpers ------------------------------
                # cell = kt*2 + parity; 16 segments of 2 cells
                SEGS = [(s, s + 2) for s in range(0, 32, 2)]

                def emit_score_seg(j, qc, c0, c1, Pu):
                    n = c1 - c0
                    sp = spool.tile([P, 2 * QC], F32, tag="S")
                    for u, cell in enumerate(range(c0, c1)):
                        kt, par = cell >> 1, cell & 1
                        off = par * HD
                        nc.tensor.matmul(
                            sp[:, u * QC : (u + 1) * QC],
                            KT[off : off + HD, j, kt * P : (kt + 1) * P],
                            QT[off : off + HD, j, ts(qc, QC)],
                            start=True,
                            stop=True,
                        )
                    nc.scalar.activation(
                        out=Pu[:, c0:c1, :].rearrange("p a b -> p (a b)"),
                        in_=sp[:, 0 : n * QC],
                        func=EXP,
                        scale=scale,
                    )

                def emit_av(j, par, opsum, Pu, kt):
                    h = 2 * j + par
                    nc.tensor.matmul(
                        opsum[0 : HD + 1, :],
                        V[:, kt, h, :],
                        Pu[:, 2 * kt + par, :],
                        start=(kt == 0),
                        stop=(kt == NKT - 1),
                    )

                def emit_finish(opsum, j, par, qc):
                    off = par * HD
                    den = nrmp.tile([1, QC], F32, tag="dn")
                    nc.vector.tensor_copy(den[:], opsum[HD : HD + 1, :])
                    recip = nrmp.tile([1, QC], F32, tag="rc")
                    nc.vector.reciprocal_approx_fast(recip[:], den[:])
                    bcs = nrmp.tile([HD, QC], F32, tag="bcs")
                    nc.gpsimd.partition_broadcast(bcs[:], recip[:], channels=HD)
                    nc.vector.tensor_mul(
                        outT[off : off + HD, j, ts(qc, QC)],
                        opsum[0:HD, :],
                        bcs[:],
                    )

                # ---------- static filler schedule ----------------------------
                # unit u = (qc, j) with qc outer: u = qc*4 + j.
                # Constraints encoded here: KT(et_j)/QT(et_j, qc) chains are
                # emitted at least one unit before the unit that consumes
                # them; V th0 before unit 1's early AV, V th1 within unit 1;
                # y(tcq) only after all 4 pairs of that tcq have finished
                # (AV of (tcq, j3) runs in the next unit).
                FILLER = {
                    0: [("kt", 0, 1, 0), ("kt", 0, 1, 1),
                        ("kt", 1, 0, 0), ("kt", 1, 0, 1), ("kt", 1, 1, 0),
                        ("kt", 1, 1, 1), ("qt", 1, 0, 0),
                        ("v", 0), ("v", 1), ("v", 2), ("v", 3),
                        ("v", 4), ("v", 5), ("v", 6), ("v", 7)],
                    1: [("v", 8), ("v", 9), ("v", 10), ("v", 11),
                        ("v", 12), ("v", 13), ("v", 14), ("v", 15),
                        ("kt", 2, 0, 0), ("kt", 2, 0, 1), ("kt", 2, 1, 0),
                        ("kt", 2, 1, 1), ("qt", 2, 0, 0)],
                    2: [("kt", 3, 0, 0), ("kt", 3, 0, 1), ("kt", 3, 1, 0),
                        ("kt", 3, 1, 1), ("qt", 3, 0, 0), ("qt", 0, 0, 1)],
                    3: [("qt", 1, 0, 1), ("qt", 2, 0, 1), ("qt", 3, 0, 1)],
                    4: [("qt", 0, 1, 0), ("qt", 1, 1, 0)],
                    5: [("qt", 2, 1, 0), ("qt", 3, 1, 0),
                        ("y", 0, 0), ("y", 1, 0)],
                    6: [("qt", 0, 1, 1), ("qt", 1, 1, 1),
                        ("y", 2, 0), ("y", 3, 0), ("y", 4, 0)],
                    7: [("qt", 2, 1, 1), ("qt", 3, 1, 1),
                        ("y", 5, 0), ("y", 6, 0), ("y", 7, 0)],
                    8: [],
                    9: [("y", 0, 1), ("y", 1, 1), ("y", 2, 1)],
                    10: [("y", 3, 1), ("y", 4, 1), ("y", 5, 1)],
                    11: [("y", 6, 1), ("y", 7, 1)],
                    12: [],
                    13: [("y", 0, 2), ("y", 1, 2), ("y", 2, 2)],
                    14: [("y", 3, 2), ("y", 4, 2), ("y", 5, 2),
                         ("y", 6, 2), ("y", 7, 2)],
                    15: [],
                }

                def emit_filler(item):
                    kind = item[0]
                    if kind == "kt":
                        kt_chain(*item[1:])
                    elif kind == "qt":
                        qt_chain(*item[1:])
                    elif kind == "v":
                        v_chain(item[1])
                    else:
                        y_group(item[1], item[2])

                # ---------- lead-in: only what unit 0's first cells need ------
                # (cells 0..15 are kt 0-7 = th0 keys; th1 KT chains follow in
                # unit 0's filler before cell 16 is reached)
                kt_chain(0, 0, 0)
                kt_chain(0, 0, 1)
                qt_chain(0, 0, 0)

                # ---------- the unit loop -------------------------------------
                units = [(qc, j) for qc in range(NQC) for j in range(NP)]
                # Segments are emitted in PAIRS (both psum slots fill
                # back-to-back) so the e/o score alternation is unbroken for
                # 4 cells and only one full-drain wait is paid per 4 cells.
                NDS = len(SEGS) // 2
                # AV-MM spread across the 8 double-steps (32 per unit)
                SPREAD = [4] * NDS
                # unit 1 consumes V-th1 chains first, AV back-loaded
                SPREAD_U1 = [0, 0, 6, 6, 6, 6, 4, 4]

                prev = None  # (opsum_e, opsum_o, Pu, j, qc)
                last_ui = len(units) - 1
                self_av = 0
                for ui, (qc, j) in enumerate(units):
                    Pu = pbuf.tile([P, 2 * NKT, QC], MDT, tag="P")
                    if ui == last_ui:
                        # the last unit's AV trails its own exps inside the
                        # segment loop; its accumulators live in the (now
                        # idle) filler psum pool since ovpool's banks are
                        # still held by the previous unit's AV.
                        own_e = p1pool.tile([P, QC], F32, tag="pp")
                        own_o = p1pool.tile([P, QC], F32, tag="pp")
                    filler = list(FILLER.get(ui, []))
                    # split filler across segments roughly evenly
                    nf = len(filler)
                    spread = SPREAD_U1 if ui == 1 else SPREAD
                    av_i = 0
                    for si in range(NDS):
                        emit_score_seg(j, qc, 4 * si, 4 * si + 2, Pu)
                        emit_score_seg(j, qc, 4 * si + 2, 4 * si + 4, Pu)
                        if prev is not None:
                            pe, po, pPu, pj, pqc = prev
                            for _ in range(spread[si]):
                                if av_i >= 2 * NKT:
                                    break
                                kt, par = av_i >> 1, av_i & 1
                                emit_av(pj, par, pe if par == 0 else po,
                                        pPu, kt)
                                av_i += 1
                        if ui == last_ui:
                            while self_av < min(4 * (si - 1), 2 * NKT):
                                kt, par = self_av >> 1, self_av & 1
                                emit_av(j, par, own_e if par == 0 else own_o,
                                        Pu, kt)
                                self_av += 1
                        # one filler chain after each segment, round-robin
                        want = (nf * (si + 1)) // NDS
                        while len(filler) > nf - want and filler:
                            emit_filler(filler.pop(0))
                    if prev is not None:
                        pe, po, pPu, pj, pqc = prev
                        while av_i < 2 * NKT:
                            kt, par = av_i >> 1, av_i & 1
                            emit_av(pj, par, pe if par == 0 else po, pPu, kt)
                            av_i += 1
                        emit_finish(pe, pj, 0, pqc)
                        emit_finish(po, pj, 1, pqc)
                    if ui != last_ui:
                        opsum_e = ovpool.tile([P, QC], F32, tag="ov")
                        opsum_o = ovpool.tile([P, QC], F32, tag="ov")
                        prev = (opsum_e, opsum_o, Pu, j, qc)
                    else:
                        prev = (own_e, own_o, Pu, j, qc)

                # ---------- tail ----------------------------------------------
                pe, po, pPu, pj, pqc = prev
                while self_av < 2 * NKT:
                    kt, par = self_av >> 1, self_av & 1
                    emit_av(pj, par, pe if par == 0 else po, pPu, kt)
                    self_av += 1
                emit_finish(pe, pj, 0, pqc)
                emit_finish(po, pj, 1, pqc)
                for et in range(DM // P):
                    y_group(et, 3)

    nc.compile()
    return nc


_CACHE: dict = {}


def _get_program(scale: float):
    key = round(float(scale), 12)
    if key not in _CACHE:
        _CACHE[key] = build(key)
    return _CACHE[key]


def _make_in_maps(x, w_qkv, w_out):
    cdt = np.float16
    xTs = [np.ascontiguousarray(x[b].T).astype(cdt) for b in range(4)]
    wslices = []
    for hg in range(2):
        sl = slice(hg * E, (hg + 1) * E)
        wslices.append(
            {
                "wq": np.ascontiguousarray(w_qkv[0 * DM :][sl, :].T).astype(cdt),
                "wk": np.ascontiguousarray(w_qkv[1 * DM :][sl, :].T).astype(cdt),
                "wv": np.ascontiguousarray(w_qkv[2 * DM :][sl, :].T).astype(cdt),
                "wo": np.ascontiguousarray(w_out[:, sl].T).astype(cdt),
            }
        )
    in_maps = []
    for c in range(8):
        b, hg = c // 2, c % 2
        m = {"xT": xTs[b]}
        m.update(wslices[hg])
        in_maps.append(m)
    return in_maps


def _execute(x, w_qkv, w_out, rescale, **spmd_kwargs):
    scale = float(np.asarray(rescale)) / math.sqrt(HD)
    nc = _get_program(scale)
    in_maps = _make_in_maps(x, w_qkv, w_out)
    return run_bass_kernel_spmd(nc, in_maps, list(range(8)), **spmd_kwargs)


def kernel(x, w_qkv, w_out, b_out, rescale):
    x = np.asarray(x, dtype=np.float32)
    w_qkv = np.asarray(w_qkv, dtype=np.float32)
    w_out = np.asarray(w_out, dtype=np.float32)
    b_out = np.asarray(b_out, dtype=np.float32)
    res = _execute(x, w_qkv, w_out, rescale).results
    y = np.empty((4, T, DM), dtype=np.float32)
    for b in range(4):
        acc = res[2 * b]["yT"].astype(np.float32) + res[2 * b + 1]["yT"].astype(
            np.float32
        )
        y[b] = acc.T + b_out
    return y


# revision 28
# speedup vs baseline: 1.1060x; 1.0024x over previous
"""DriftAwareMultiHeadAttention on 8 Trainium2 NeuronCores.

Sharding (per spec hint): core c -> (batch b = c//2, head-group hg = c%2).
Each core runs the QKV projection column-parallel over its 8 heads, full
attention for those heads, and a row-parallel partial output projection.
Host gather: y[b] = (yT[2b] + yT[2b+1]).T + b_out.

Paired-head phase 2.  The per-core 8 heads are processed as 4 pairs
(2j, 2j+1); the pair's K^T/Q^T live in the same e-tile at partition
offsets 0/64, so their K=64 score matmuls carry tile_position (0,0) and
(64,0) (auto-derived from base_partition) and run CONCURRENTLY on
disjoint row-groups of the PE array.  Measured: within a quad
[e,o,e,o] the e->o gap is ~3ns.  Caveat: a row-tiled LDWEIGHTS cannot
background-load while its row-group streams, so the o->e and quad-exit
transitions pay ~+100ns each -- scores land at ~82us (vs 114 unpaired).

Device layout is feature-on-partition / token-on-free throughout:
  - Q^T, K^T: [512, 2048] fp16 (pair j -> e-tile j, head parity ->
    partition offset 0/64).  V: [tokens, 8 heads x (64+1)] fp16 -- the
    extra "ones" column makes the AV matmul emit the softmax denominator
    in psum row 64 for free (it is what forces AV to M=65 and blocks
    col-packing two heads' AV; GpSimd cannot read PSUM and DVE partition
    reduction is too slow, so the ones-column is the only cheap path).
  - Unit (qc, j): 32 cells (cell = kt*2 + parity) of S^T score matmuls.
    Segment pairs (2 psum tiles x 2 cells, filled back-to-back) -> one
    flat-2D-AP exp per segment (ScalarE, scale folded in) -> Pu cells
    [128, 32, 512] fp16.  Flat 2D APs on the exp are load-bearing:
    3D/strided APs cost ~2x ACT overhead AND degrade PE issue spacing
    (259 -> 216 ns/matmul when flattened).
  - AV accumulates [65, 512] fp32 over 16 k-tiles, one bank per head.
    Row 64 -> DVE reciprocal_approx_fast + GpSimd partition_broadcast +
    DVE multiply into outT.  The last unit's AV trails its own exps
    inside the segment loop (accumulators borrowed from the filler
    pool) to shorten the tail.
  - output projection y^T = wo^T @ outT in fp16 with fp32 psum, fp16
    yT output (host promotes); emitted as 4-matmul groups.

Phase-1 projections are folded into the phase-2 unit stream as filler
chains in a dedicated 2-bank psum pool (never touching the score-psum
rotation): only K^T(et0,th0) + Q^T(et0,qc0) gate unit 0, everything
else (V, remaining K^T/Q^T chains, y-groups) fills the PE while ScalarE
drains exps.  Input DMAs are chunk-interleaved (wk/x0/wq) and spread
over the scalar/sync/gpsimd DGE queues so the first exp starts ~20us in.

PSUM budget: scores 2x[128,2x512] (4) + filler 2 + AV 2 = 8 banks.
fp16 for all 16-bit operands (same matmul speed as bf16, 8x lower
rounding error on these O(1) tensors); fp32 psum.  rel err ~8e-4.
"""

import math

import numpy as np

import concourse.bass as bass
import concourse.mybir as mybir
import concourse.tile as tile
from concourse import bacc
from concourse.bass import ds, ts
from concourse.bass_utils import run_bass_kernel_spmd

P = 128
T = 2048        # tokens per batch
DM = 1024       # model dim
E = 512         # per-core projection width (8 heads * 64)
H = 8           # heads per core
HD = 64
CD = DM // P    # contraction chunks over model dim
NKT = T // P    # k tiles per head
QC = 512        # q chunk
NQC = T // QC
TH = T // 2
NP = H // 2     # head pairs
F32 = mybir.dt.float32
FP16 = mybir.dt.float16
EXP = mybir.ActivationFunctionType.Exp


def build(scale: float):
    MDT = FP16
    nc = bacc.Bacc(None, target_bir_lowering=False, debug=False)
    xT = nc.declare_dram_parameter("xT", [DM, T], MDT, isOutput=False)
    wq = nc.declare_dram_parameter("wq", [DM, E], MDT, isOutput=False)
    wk = nc.declare_dram_parameter("wk", [DM, E], MDT, isOutput=False)
    wv = nc.declare_dram_parameter("wv", [DM, E], MDT, isOutput=False)
    wo = nc.declare_dram_parameter("wo", [E, DM], MDT, isOutput=False)
    yT = nc.declare_dram_parameter("yT", [DM, T], MDT, isOutput=True)

    with tile.TileContext(nc) as tc:
        with (
            tc.tile_pool(name="qk", bufs=1) as qkp,
            tc.tile_pool(name="vp", bufs=1) as vp,
            tc.tile_pool(name="misc", bufs=1) as miscp,
            tc.tile_pool(name="wts", bufs=1) as wp,
            tc.tile_pool(name="xt", bufs=1) as xp,
        ):
            QT = qkp.tile([P, 4, T], MDT, tag="QT")
            KT = qkp.tile([P, 4, T], MDT, tag="KT")
            V = vp.tile([P, NKT, H, HD + 1], MDT, tag="V")
            nc.vector.memset(V[:, :, :, HD : HD + 1], 1.0)
            # preload the exp table set so the first real exp doesn't stall
            warm = miscp.tile([1, 8], F32, tag="warm")
            nc.vector.memset(warm[:], 0.0)
            nc.scalar.activation(out=warm[:], in_=warm[:], func=EXP, scale=1.0)

            wq_sb = wp.tile([P, CD, E], MDT, tag="wq")
            wk_sb = wp.tile([P, CD, E], MDT, tag="wk")
            wv_sb = wp.tile([P, CD, E], MDT, tag="wv")
            # DMA order is the critical path to the first exp: the first
            # unit's cells 0..15 touch only th0 tokens, so KT(et0,th0) +
            # QT(et0,th0,tcl0) gate it -> wk, x(th0), wq first; x(th1), wv
            # after.
            xts = {}

            def _dma_x(th):
                for c in range(CD):
                    xt = xp.tile([P, TH], MDT, tag=f"x{th}_{c}")
                    nc.sync.dma_start(
                        out=xt[:],
                        in_=xT[c * P : (c + 1) * P, th * TH : (th + 1) * TH],
                    )
                    xts[(th, c)] = xt

            # Interleave wk/x(th0) chunk DMAs so the first KT chain's matmul
            # for chunk c can start as soon as chunk c lands, and spread the
            # input streams across several engines' DGE queues so the ramp
            # isn't serialized behind one queue.
            wk_r = wk.rearrange("(c p) e -> p c e", p=P)
            wq_r = wq.rearrange("(c p) e -> p c e", p=P)
            for c in range(CD):
                nc.scalar.dma_start(out=wk_sb[:, c, :], in_=wk_r[:, c, :])
                xt = xp.tile([P, TH], MDT, tag=f"x0_{c}")
                nc.sync.dma_start(
                    out=xt[:], in_=xT[c * P : (c + 1) * P, 0:TH]
                )
                xts[(0, c)] = xt
                nc.gpsimd.dma_start(out=wq_sb[:, c, :], in_=wq_r[:, c, :])
            _dma_x(1)
            nc.sync.dma_start(out=wv_sb[:], in_=wv.rearrange("(c p) e -> p c e", p=P))

            with (
                tc.tile_pool(name="outp", bufs=1) as outp,
                tc.tile_pool(name="pbuf", bufs=2) as pbuf,
                tc.tile_pool(name="nrm", bufs=2) as nrmp,
                tc.tile_pool(name="wop", bufs=1) as wop,
                tc.tile_pool(name="yev", bufs=3) as yev,
                tc.tile_pool(name="sps", bufs=2, space="PSUM") as spool,
                tc.tile_pool(name="ovp", bufs=2, space="PSUM") as ovpool,
                tc.tile_pool(name="p1", bufs=2, space="PSUM") as p1pool,
            ):
                outT = outp.tile([P, 4, T], MDT, tag="outT")
                wo_sb = wop.tile([P, 4, DM], MDT, tag="wo")
                nc.sync.dma_start(
                    out=wo_sb[:], in_=wo.rearrange("(c p) e -> p c e", p=P)
                )

                # ---------- filler chain emitters (each: one 8/4-MM chain) ----
                # Filler runs in its own double-buffered 2-bank psum pool so
                # it never perturbs the score-segment double-buffering.
                # Total psum: scores 2x2 + filler 2 + AV 2 = 8 banks.
                def kt_chain(et, th, tcl):
                    ps = p1pool.tile([P, QC], F32, tag="pp")
                    for c in range(CD):
                        nc.tensor.matmul(
                            ps[:],
                            wk_sb[:, c, ts(et, P)],
                            xts[(th, c)][:, ts(tcl, QC)],
                            start=(c == 0),
                            stop=(c == CD - 1),
                        )
                    nc.vector.tensor_copy(
                        KT[:, et, ds(th * TH + tcl * QC, QC)], ps[:]
                    )

                def qt_chain(et, th, tcl):
                    ps = p1pool.tile([P, QC], F32, tag="pp")
                    for c in range(CD):
                        nc.tensor.matmul(
                            ps[:],
                            wq_sb[:, c, ts(et, P)],
                            xts[(th, c)][:, ts(tcl, QC)],
                            start=(c == 0),
                            stop=(c == CD - 1),
                        )
                    nc.vector.tensor_copy(
                        QT[:, et, ds(th * TH + tcl * QC, QC)], ps[:]
                    )

                def v_chain(tt):
                    th, ttl = tt // (TH // P), tt % (TH // P)
                    ps = p1pool.tile([P, E], F32, tag="pp")
                    for c in range(CD):
                        nc.tensor.matmul(
                            ps[:],
                            xts[(th, c)][:, ts(ttl, P)],
                            wv_sb[:, c, :],
                            start=(c == 0),
                            stop=(c == CD - 1),
                        )
                    nc.vector.tensor_copy(
                        V[:, tt, :, 0:HD],
                        ps[:].rearrange("p (h e) -> p h e", h=H),
                    )

                def y_group(et, tcq):
                    ps = p1pool.tile([P, QC], F32, tag="pp")
                    for fc in range(4):
                        nc.tensor.matmul(
                            ps[:],
                            wo_sb[:, fc, ts(et, P)],
                            outT[:, fc, ts(tcq, QC)],
                            start=(fc == 0),
                            stop=(fc == 3),
                        )
                    yt = yev.tile([P, QC], MDT, tag="ye")
                    nc.vector.tensor_copy(yt[:], ps[:])
                    nc.sync.dma_start(
                        out=yT[et * P : (et + 1) * P, ts(tcq, QC)], in_=yt[:]
                    )

                # ---------- phase-2 unit helpers ------------------------------
                # cell = kt*2 + parity; 16 segments of 2 cells
                SEGS = [(s, s + 2) for s in range(0, 32, 2)]

                def emit_score_seg(j, qc, c0, c1, Pu):
                    n = c1 - c0
                    sp = spool.tile([P, 2 * QC], F32, tag="S")
                    for u, cell in enumerate(range(c0, c1)):
                        kt, par = cell >> 1, cell & 1
                        off = par * HD
                        nc.tensor.matmul(
                            sp[:, u * QC : (u + 1) * QC],
                            KT[off : off + HD, j, kt * P : (kt + 1) * P],
                            QT[off : off + HD, j, ts(qc, QC)],
                            start=True,
                            stop=True,
                        )
                    nc.scalar.activation(
                        out=Pu[:, c0:c1, :].rearrange("p a b -> p (a b)"),
                        in_=sp[:, 0 : n * QC],
                        func=EXP,
                        scale=scale,
                    )

                def emit_av(j, par, opsum, Pu, kt):
                    h = 2 * j + par
                    nc.tensor.matmul(
                        opsum[0 : HD + 1, :],
                        V[:, kt, h, :],
                        Pu[:, 2 * kt + par, :],
                        start=(kt == 0),
                        stop=(kt == NKT - 1),
                    )

                def emit_finish(opsum, j, par, qc):
                    off = par * HD
                    den = nrmp.tile([1, QC], F32, tag="dn")
                    nc.vector.tensor_copy(den[:], opsum[HD : HD + 1, :])
                    recip = nrmp.tile([1, QC], F32, tag="rc")
                    nc.vector.reciprocal_approx_fast(recip[:], den[:])
                    bcs = nrmp.tile([HD, QC], F32, tag="bcs")
                    nc.gpsimd.partition_broadcast(bcs[:], recip[:], channels=HD)
                    nc.vector.tensor_mul(
                        outT[off : off + HD, j, ts(qc, QC)],
                        opsum[0:HD, :],
                        bcs[:],
                    )

                # ---------- static filler schedule ----------------------------
                # unit u = (qc, j) with qc outer: u = qc*4 + j.
                # Constraints encoded here: KT(et_j)/QT(et_j, qc) chains are
                # emitted at least one unit before the unit that consumes
                # them; V th0 before unit 1's early AV, V th1 within unit 1;
                # y(tcq) only after all 4 pairs of that tcq have finished
                # (AV of (tcq, j3) runs in the next unit).
                FILLER = {
                    0: [("kt", 0, 1, 0), ("kt", 0, 1, 1),
                        ("kt", 1, 0, 0), ("kt", 1, 0, 1), ("kt", 1, 1, 0),
                        ("kt", 1, 1, 1), ("qt", 1, 0, 0),
                        ("v", 0), ("v", 1), ("v", 2), ("v", 3),
                        ("v", 4), ("v", 5), ("v", 6), ("v", 7)],
                    1: [("v", 8), ("v", 9), ("v", 10), ("v", 11),
                        ("v", 12), ("v", 13), ("v", 14), ("v", 15),
                        ("kt", 2, 0, 0), ("kt", 2, 0, 1), ("kt", 2, 1, 0),
                        ("kt", 2, 1, 1), ("qt", 2, 0, 0)],
                    2: [("kt", 3, 0, 0), ("kt", 3, 0, 1), ("kt", 3, 1, 0),
                        ("kt", 3, 1, 1), ("qt", 3, 0, 0), ("qt", 0, 0, 1)],
                    3: [("qt", 1, 0, 1), ("qt", 2, 0, 1), ("qt", 3, 0, 1)],
                    4: [("qt", 0, 1, 0), ("qt", 1, 1, 0)],
                    5: [("qt", 2, 1, 0), ("qt", 3, 1, 0),
                        ("y", 0, 0), ("y", 1, 0)],
                    6: [("qt", 0, 1, 1), ("qt", 1, 1, 1),
                        ("y", 2, 0), ("y", 3, 0), ("y", 4, 0)],
                    7: [("qt", 2, 1, 1), ("qt", 3, 1, 1),
                        ("y", 5, 0), ("y", 6, 0), ("y", 7, 0)],
                    8: [],
                    9: [("y", 0, 1), ("y", 1, 1), ("y", 2, 1)],
                    10: [("y", 3, 1), ("y", 4, 1), ("y", 5, 1)],
                    11: [("y", 6, 1), ("y", 7, 1)],
                    12: [],
                    13: [("y", 0, 2), ("y", 1, 2), ("y", 2, 2)],
                    14: [("y", 3, 2), ("y", 4, 2), ("y", 5, 2),
                         ("y", 6, 2), ("y", 7, 2)],
                    15: [],
                }

                def emit_filler(item):
                    kind = item[0]
                    if kind == "kt":
                        kt_chain(*item[1:])
                    elif kind == "qt":
                        qt_chain(*item[1:])
                    elif kind == "v":
                        v_chain(item[1])
                    else:
                        y_group(item[1], item[2])

                # ---------- lead-in: only what unit 0's first cells need ------
                # (cells 0..15 are kt 0-7 = th0 keys; th1 KT chains follow in
                # unit 0's filler before cell 16 is reached)
                kt_chain(0, 0, 0)
                kt_chain(0, 0, 1)
                qt_chain(0, 0, 0)

                # ---------- the unit loop -------------------------------------
                units = [(qc, j) for qc in range(NQC) for j in range(NP)]
                # Segments are emitted in PAIRS (both psum slots fill
                # back-to-back) so the e/o score alternation is unbroken for
                # 4 cells and only one full-drain wait is paid per 4 cells.
                NDS = len(SEGS) // 2
                # AV-MM spread across the 8 double-steps (32 per unit).
                # Front-loaded for mid/late units so the previous unit's Pu
                # buffer frees ~2 double-steps earlier (unit-boundary exp
                # stalls of ~1.5-2us observed with a flat spread); early
                # units keep a flat/back-loaded spread because ACT still
                # lags the PE there and early AV would stall in-order PE.
                SPREAD = [4] * NDS
                SPREAD_FRONT = [6, 6, 4, 4, 4, 4, 2, 2]
                # unit 1 consumes V-th1 chains first, AV back-loaded
                SPREAD_U1 = [0, 0, 6, 6, 6, 6, 4, 4]

                prev = None  # (opsum_e, opsum_o, Pu, j, qc)
                last_ui = len(units) - 1
                self_av = 0
                for ui, (qc, j) in enumerate(units):
                    Pu = pbuf.tile([P, 2 * NKT, QC], MDT, tag="P")
                    if ui == last_ui:
                        # the last unit's AV trails its own exps inside the
                        # segment loop; its accumulators live in the (now
                        # idle) filler psum pool since ovpool's banks are
                        # still held by the previous unit's AV.
                        own_e = p1pool.tile([P, QC], F32, tag="pp")
                        own_o = p1pool.tile([P, QC], F32, tag="pp")
                    filler = list(FILLER.get(ui, []))
                    # split filler across segments roughly evenly
                    nf = len(filler)
                    if ui == 1:
                        spread = SPREAD_U1
                    elif ui >= 4:
                        spread = SPREAD_FRONT
                    else:
                        spread = SPREAD
                    av_i = 0
                    for si in range(NDS):
                        emit_score_seg(j, qc, 4 * si, 4 * si + 2, Pu)
                        emit_score_seg(j, qc, 4 * si + 2, 4 * si + 4, Pu)
                        if prev is not None:
                            pe, po, pPu, pj, pqc = prev
                            for _ in range(spread[si]):
                                if av_i >= 2 * NKT:
                                    break
                                kt, par = av_i >> 1, av_i & 1
                                emit_av(pj, par, pe if par == 0 else po,
                                        pPu, kt)
                                av_i += 1
                            if av_i == 2 * NKT:
                                # finish as soon as the AV chain completes so
                                # Pu/opsum free early and outT is ready for
                                # y-groups sooner
                                emit_finish(pe, pj, 0, pqc)
                                emit_finish(po, pj, 1, pqc)
                                av_i += 1  # mark finishes done
                        if ui == last_ui:
                            while self_av < min(4 * (si - 1), 2 * NKT):
                                kt, par = self_av >> 1, self_av & 1
                                emit_av(j, par, own_e if par == 0 else own_o,
                                        Pu, kt)
                                self_av += 1
                        # one filler chain after each segment, round-robin
                        want = (nf * (si + 1)) // NDS
                        while len(filler) > nf - want and filler:
                            emit_filler(filler.pop(0))
                    if prev is not None and av_i <= 2 * NKT:
                        pe, po, pPu, pj, pqc = prev
                        while av_i < 2 * NKT:
                            kt, par = av_i >> 1, av_i & 1
                            emit_av(pj, par, pe if par == 0 else po, pPu, kt)
                            av_i += 1
                        emit_finish(pe, pj, 0, pqc)
                        emit_finish(po, pj, 1, pqc)
                    if ui != last_ui:
                        opsum_e = ovpool.tile([P, QC], F32, tag="ov")
                        opsum_o = ovpool.tile([P, QC], F32, tag="ov")
                        prev = (opsum_e, opsum_o, Pu, j, qc)
                    else:
                        prev = (own_e, own_o, Pu, j, qc)

                # ---------- tail ----------------------------------------------
                pe, po, pPu, pj, pqc = prev
                while self_av < 2 * NKT:
                    kt, par = self_av >> 1, self_av & 1
                    emit_av(pj, par, pe if par == 0 else po, pPu, kt)
                    self_av += 1
                emit_finish(pe, pj, 0, pqc)
                emit_finish(po, pj, 1, pqc)
                for et in range(DM // P):
                    y_group(et, 3)

    nc.compile()
    return nc


_CACHE: dict = {}


def _get_program(scale: float):
    key = round(float(scale), 12)
    if key not in _CACHE:
        _CACHE[key] = build(key)
    return _CACHE[key]


def _make_in_maps(x, w_qkv, w_out):
    cdt = np.float16
    xTs = [np.ascontiguousarray(x[b].T).astype(cdt) for b in range(4)]
    wslices = []
    for hg in range(2):
        sl = slice(hg * E, (hg + 1) * E)
        wslices.append(
            {
                "wq": np.ascontiguousarray(w_qkv[0 * DM :][sl, :].T).astype(cdt),
                "wk": np.ascontiguousarray(w_qkv[1 * DM :][sl, :].T).astype(cdt),
                "wv": np.ascontiguousarray(w_qkv[2 * DM :][sl, :].T).astype(cdt),
                "wo": np.ascontiguousarray(w_out[:, sl].T).astype(cdt),
            }
        )
    in_maps = []
    for c in range(8):
        b, hg = c // 2, c % 2
        m = {"xT": xTs[b]}
        m.update(wslices[hg])
        in_maps.append(m)
    return in_maps


def _execute(x, w_qkv, w_out, rescale, **spmd_kwargs):
    scale = float(np.asarray(rescale)) / math.sqrt(HD)
    nc = _get_program(scale)
    in_maps = _make_in_maps(x, w_qkv, w_out)
    return run_bass_kernel_spmd(nc, in_maps, list(range(8)), **spmd_kwargs)


def kernel(x, w_qkv, w_out, b_out, rescale):
    x = np.asarray(x, dtype=np.float32)
    w_qkv = np.asarray(w_qkv, dtype=np.float32)
    w_out = np.asarray(w_out, dtype=np.float32)
    b_out = np.asarray(b_out, dtype=np.float32)
    res = _execute(x, w_qkv, w_out, rescale).results
    y = np.empty((4, T, DM), dtype=np.float32)
    for b in range(4):
        acc = res[2 * b]["yT"].astype(np.float32) + res[2 * b + 1]["yT"].astype(
            np.float32
        )
        y[b] = acc.T + b_out
    return y
